# revision 40
# baseline (speedup 1.0000x reference)
"""Trainium2 Bass kernel for BilinearInteractionPlusLayer.

Math (per batch row b):
    pairs (i,j), i<j over F=40 fields, P=C(40,2)=780 pairs
    t[b,p,f] = sum_e x[b,i,e] * W[p,e,f]
    q[b,p]   = sum_f t[b,p,f] * x[b,j,f]
    h[b,d]   = sum_p q[b,p] * dense_w[p,d] + dense_b[d]
    out      = LayerNorm(h) * gamma + beta          (eps = 1e-3)

Sharding: data-parallel over batch, 2048 -> 256 rows on each of 8 cores.
W / dense_w / LN params are replicated. No collectives.

Per-core pipeline, pair math in a transposed "[feature x batch]" layout:
  - x arrives host-packed as bf16: xT[32*(j%4)+f, (j//4)*256 + b].
  - One "chunk" = (field i, j-group jg): stage-1 matmul
        lhsT = W chunk [32e x 128(c,f)] at row strip r = i%4
        rhs  = xT_i    [32e x 256b]
        out  = t chunk [128(c,f) x 256b] in its OWN PSUM bank
    Rounds of 4 chunks (strips 0..3) issue back-to-back; distinct row
    strips + distinct PSUM banks -> the 4 matmuls run concurrently in
    the PE array (row tiling).
  - Hadamard m = t * xT over three engine routes, balanced:
        A: ScalarE cast fp32->bf16, VectorE bf16 mul (2x mode)
        B: VectorE mul direct from PSUM (fp32 in, bf16 out)
        C: ScalarE cast, GpSimd bf16 mul
  - dw2 matmuls fold dense_w and the (pair,f) reduction:
        h4[32g+d, b] += dw2_k.T @ m_k     (g = row strip of chunk k)
    4-way column tiling -> 4 concurrent accumulating matmuls.
  - dw2 operand is built on-chip from a 27KB host tensor via 4
    partition-broadcast DMAs (instead of DMAing the 860KB expansion).
  - Tail: selector matmul + PE transpose + bn_stats LayerNorm.
"""

import itertools

import numpy as np

import concourse.bass as bass
from concourse import bacc, mybir
from concourse.bass_utils import run_bass_kernel_spmd
from concourse.tile import TileContext

F32 = mybir.dt.float32
BF16 = mybir.dt.bfloat16
NP_BF16 = mybir.dt.np(BF16)

B, F, E, P, D = 2048, 40, 32, 780, 16
NCORES = 8
BS = B // NCORES          # 256 batch rows per core
NJG = F // 4              # 10 j-groups of 4 fields
LN_EPS = 1e-3

# Hadamard route weights per PAIR-group of 2 chunks
# (A: cast+DVE 2x, B: DVE direct fp32-from-PSUM, C: cast+gpsimd)
ROUTE_W = (28, 42, 35)

# chunks moved to strip 3 to even the per-strip counts; each moved
# chunk's x_i is duplicated at partition strip 3 in an extra xT block.
# (empty: rebalancing measured net-neutral vs. scheduling noise)
MOVES = []
DUP_FIELDS = sorted({i for i, _ in MOVES})       # fields duplicated at strip 3
DUP_BLK = {i: NJG + m for m, i in enumerate(DUP_FIELDS)}
NXB = NJG + len(DUP_FIELDS)         # xT free blocks incl. duplicates


def _make_chunks():
    """One chunk = (field i, j-group jg): a [32e x 128(c,f)] stage-1 matmul.
    s = slot index within the chunk's row strip; xblk = xT free block
    holding x_i at partition strip r."""
    chunks = []
    per_strip = [0, 0, 0, 0]
    for i in range(F):
        for jg in range((i + 1) // 4, NJG):
            if (i, jg) in MOVES:
                r = 3
                xblk = DUP_BLK[i]
            else:
                r = i % 4
                xblk = i // 4
            chunks.append(
                {"i": i, "jg": jg, "r": r, "s": per_strip[r],
                 "k": len(chunks), "xblk": xblk}
            )
            per_strip[r] += 1
    return chunks, per_strip


CHUNKS, PER_STRIP = _make_chunks()
NCH = len(CHUNKS)          # 210
SLOTS = max(PER_STRIP)     # 53 rounds
CH_BY_RS = {(c["r"], c["s"]): c for c in CHUNKS}


NPAIRS = sum((c + 1) // 2 for c in PER_STRIP)   # pair-groups of 2 slots


def _routes(n):
    """Per-group Hadamard route (0=A,1=B,2=C) via largest-remainder."""
    total = sum(ROUTE_W)
    nr = len(ROUTE_W)
    taken = [0] * nr
    out = []
    for k in range(n):
        deficits = [ROUTE_W[j] * (k + 1) / total - taken[j] for j in range(nr)]
        j = max(range(nr), key=lambda jj: deficits[jj])
        taken[j] += 1
        out.append(j)
    return out


ROUTES = _routes(NPAIRS)


def _host_weights(W, dense_w):
    """Pack W into per-strip stationary chunks and dense_w into dwpack
    [4, NCH*16]: dwpack[c, k*16+d] = dense_w[p(k,c), d] (zero if absent)."""
    pair_idx = {pq: n for n, pq in enumerate(itertools.combinations(range(F), 2))}
    wsb = np.zeros((128, SLOTS * 128), np.float32)
    dwp = np.zeros((4, NCH * 16), np.float32)
    for ch in CHUNKS:
        i, jg, r, s, k = ch["i"], ch["jg"], ch["r"], ch["s"], ch["k"]
        for c in range(4):
            j = 4 * jg + c
            if j <= i:
                continue
            p = pair_idx[(i, j)]
            wsb[32 * r:32 * r + 32, s * 128 + 32 * c:s * 128 + 32 * c + 32] = W[p]
            dwp[c, k * 16:(k + 1) * 16] = dense_w[p]
    return wsb.astype(NP_BF16), dwp.astype(NP_BF16)


def _host_xt(xc):
    """Per-core phase layout: xt[32*(j%4)+f, (j//4)*BS + b] = xc[b, j, f],
    plus duplicate blocks at strip 3 for the rebalanced chunks."""
    arr = xc.transpose(1, 2, 0)                    # [F, E, BS]
    arr = arr.reshape(NJG, 4, E, BS)               # [jg, c, f, b]
    arr = arr.transpose(1, 2, 0, 3)                # [c, f, jg, b]
    full = np.zeros((128, NXB * BS), np.float32)
    full[:, :NJG * BS] = arr.reshape(128, NJG * BS)
    for i, blk in DUP_BLK.items():
        full[96:128, blk * BS:(blk + 1) * BS] = xc[:, i, :].T
    return full.astype(NP_BF16)


def _host_ident4():
    """[128, 16] with a 16x16 identity at each 32-partition strip."""
    id4 = np.zeros((128, 16), np.float32)
    for g in range(4):
        id4[32 * g:32 * g + 16, :] = np.eye(16, dtype=np.float32)
    return id4


def _build_bass():
    nc = bacc.Bacc(trn_type="TRN2")
    xin = nc.dram_tensor("xt", [128, NXB * BS], BF16, kind="ExternalInput")
    wsb = nc.dram_tensor("wsb", [128, SLOTS * 128], BF16, kind="ExternalInput")
    dwp = nc.dram_tensor("dwp", [4, NCH * 16], BF16, kind="ExternalInput")
    vecs = nc.dram_tensor("vecs", [3, D], F32, kind="ExternalInput")
    id4 = nc.dram_tensor("ident4", [128, D], F32, kind="ExternalInput")
    out = nc.dram_tensor("out", [BS, D], F32, kind="ExternalOutput")

    with TileContext(nc) as tc:
        with (
            tc.tile_pool(name="const", bufs=1) as const,
            tc.tile_pool(name="cast", bufs=8) as cbuf,
            tc.tile_pool(name="mbuf", bufs=12) as mbuf,
            tc.tile_pool(name="tsegp", bufs=7, space="PSUM") as tsegp,
            tc.tile_pool(name="hpsp", bufs=1, space="PSUM") as hpsp,
            tc.tile_pool(name="lnp", bufs=2) as lnp,
        ):
            # ---- constants / inputs (first pieces sized so window 0 can
            # start as early as possible; spread across issue queues so
            # descriptor generation isn't serialized on one sequencer)
            xT = const.tile([128, NXB, BS], BF16)
            nc.sync.dma_start(out=xT[:], in_=xin[:, :])
            wsb_t = const.tile([128, SLOTS * 128], BF16)
            wcuts = [0, 6 * 128, 16 * 128, 30 * 128, SLOTS * 128]
            for a, b_ in zip(wcuts[:-1], wcuts[1:]):
                nc.sync.dma_start(out=wsb_t[:, a:b_], in_=wsb[:, a:b_])
            # dw2 expansion on-chip: dw2_t[32c+f', k*16+d] = dwp[c, k*16+d]
            # broadcast down 32 partitions per c-group
            dw2_t = const.tile([128, NCH * 16], BF16)
            for c in range(4):
                src = dwp[c:c + 1, :]
                nc.sync.dma_start(
                    out=dw2_t[32 * c:32 * c + 32, :],
                    in_=bass.AP(tensor=src.tensor, offset=src.offset,
                                ap=[[0, 32]] + [list(a) for a in src.ap[1:]]),
                )
            # vecs rows: 0 = dense_b, 1 = gamma, 2 = beta
            vec_t = const.tile([128, 3, D], F32)
            src = vecs[:, :]
            nc.sync.dma_start(
                out=vec_t[:],
                in_=bass.AP(tensor=src.tensor, offset=src.offset,
                            ap=[[0, 128]] + [list(a) for a in src.ap]),
            )
            id4_t = const.tile([128, D], F32)
            nc.sync.dma_start(out=id4_t[:], in_=id4[:, :])
            eps_t = const.tile([128, 1], F32)
            nc.vector.memset(eps_t[:], LN_EPS)
            # dummy Sqrt so its activation-table load happens during the
            # DMA-bound head instead of inside the LayerNorm tail chain
            warm_t = const.tile([128, 1], F32)
            nc.scalar.activation(
                out=warm_t[:], in_=eps_t[:],
                func=mybir.ActivationFunctionType.Sqrt,
                bias=eps_t[:], scale=1.0,
            )

            # One PSUM bank holds h4 (4 col-tiled partial accumulators
            # [16d x 256b] at partition strips, free 0:256) and hsum
            # ([16d x 256b], free 256:512); ht reuses h4's range later.
            hcomb = hpsp.tile([128, 2 * BS], F32)
            h4 = hcomb[:, 0:BS]
            hsum = hcomb[0:D, BS:2 * BS]

            # ---- main pipeline over windows: window w = slots (2w, 2w+1)
            # of each strip.  Each (strip, window) owns a one-bank PSUM pair
            # tile so the 4 strips' stage-1 matmuls of a slot hit 4 distinct
            # banks and run concurrently (row tiling).
            def group_of(r, w):
                slots = [s for s in (2 * w, 2 * w + 1) if s < PER_STRIP[r]]
                return [CH_BY_RS[(r, s)] for s in slots]

            NW = (SLOTS + 1) // 2
            t_tiles = {}     # (r, w) -> psum pair tile
            m_tiles = {}     # (r, w) -> sbuf bf16 pair tile
            pair_idx = 0
            pair_route = {}  # (r, w) -> route

            def stage1(w):
                for u in range(2):
                    for r in range(4):
                        grp = group_of(r, w)
                        if u >= len(grp):
                            continue
                        if u == 0:
                            t_tiles[(r, w)] = tsegp.tile(
                                [128, 2, BS], F32, tag="t", name="tseg")
                        ch = grp[u]
                        s = ch["s"]
                        nc.tensor.matmul(
                            t_tiles[(r, w)][:, u, :],
                            lhsT=wsb_t[32 * r:32 * r + 32,
                                       s * 128:(s + 1) * 128],
                            rhs=xT[32 * r:32 * r + 32, ch["xblk"], :],
                            start=True, stop=True,
                            tile_position=(32 * r, 0),
                        )

            def hadamard(w):
                nonlocal pair_idx
                work = []
                for r in range(4):
                    grp = group_of(r, w)
                    if not grp:
                        continue
                    route = ROUTES[pair_idx]
                    pair_route[(r, w)] = route
                    pair_idx += 1
                    work.append((r, grp, route))
                for r, grp, route in work:
                    tt = t_tiles.pop((r, w))
                    m_t = mbuf.tile([128, 2, BS], BF16, tag="m")
                    m_tiles[(r, w)] = m_t
                    n = len(grp)
                    contig = n == 2 and grp[1]["jg"] == grp[0]["jg"] + 1
                    spans = ([(0, n, grp[0]["jg"])] if (contig or n == 1)
                             else [(0, 1, grp[0]["jg"]), (1, 1, grp[1]["jg"])])
                    for (u0, cnt, jg) in spans:
                        if route == 1:
                            nc.vector.tensor_mul(
                                out=m_t[:, u0:u0 + cnt, :],
                                in0=tt[:, u0:u0 + cnt, :],
                                in1=xT[:, jg:jg + cnt, :])
                        else:
                            tcast = cbuf.tile([128, 2, BS], BF16, tag="tc")
                            nc.scalar.copy(out=tcast[:, u0:u0 + cnt, :],
                                           in_=tt[:, u0:u0 + cnt, :])
                            eng = nc.vector if route == 0 else nc.gpsimd
                            eng.tensor_mul(
                                out=m_t[:, u0:u0 + cnt, :],
                                in0=tcast[:, u0:u0 + cnt, :],
                                in1=xT[:, jg:jg + cnt, :])

            def dw2_fold(w):
                for u in range(2):
                    for r in range(4):
                        grp = group_of(r, w)
                        if u >= len(grp):
                            continue
                        ch = grp[u]
                        k = ch["k"]
                        m_t = m_tiles[(r, w)]
                        nc.tensor.matmul(
                            h4[32 * r:32 * r + D, :],
                            lhsT=dw2_t[:, k * 16:(k + 1) * 16],
                            rhs=m_t[:, u, :],
                            start=(ch["s"] == 0),
                            stop=(ch["s"] == PER_STRIP[r] - 1),
                            tile_position=(0, 32 * r),
                        )
                for r in range(4):
                    m_tiles.pop((r, w), None)

            for w in range(NW + 2):
                if w < NW:
                    stage1(w)
                if 0 <= w - 1 < NW:
                    hadamard(w - 1)
                if 0 <= w - 2 < NW:
                    dw2_fold(w - 2)

            # ---- tail: combine the 4 partial h's with one selector matmul
            hg_sb = lnp.tile([128, BS], F32, tag="hgsb")
            nc.vector.memset(hg_sb[:], 0.0)
            for g in range(4):
                nc.scalar.copy(out=hg_sb[32 * g:32 * g + D, :],
                               in_=h4[32 * g:32 * g + D, :])
            nc.tensor.matmul(hsum, lhsT=id4_t[:], rhs=hg_sb[:],
                             start=True, stop=True)
            hsum_sb = lnp.tile([D, BS], F32, tag="hsum_sb")
            nc.scalar.copy(out=hsum_sb[:], in_=hsum)

            def ht_v(half):
                off = half * D
                return hcomb[0:128, off:off + D]

            for half in range(2):
                nc.tensor.transpose(
                    ht_v(half),
                    hsum_sb[:, half * 128:(half + 1) * 128],
                    id4_t[0:D, :],
                )
            # ---- LayerNorm per 128-row half
            for half in range(2):
                hb = lnp.tile([128, D], F32, tag="hb")
                nc.vector.tensor_add(out=hb[:], in0=ht_v(half),
                                     in1=vec_t[:, 0, :])
                stats = lnp.tile([128, 6], F32, tag="stats")
                nc.vector.bn_stats(out=stats[:], in_=hb[:])
                mv = lnp.tile([128, 2], F32, tag="mv")
                nc.vector.bn_aggr(out=mv[:], in_=stats[:])
                nc.scalar.activation(
                    out=mv[:, 1:2], in_=mv[:, 1:2],
                    func=mybir.ActivationFunctionType.Sqrt,
                    bias=eps_t[:], scale=1.0,
                )
                nc.vector.reciprocal(out=mv[:, 1:2], in_=mv[:, 1:2])
                nc.vector.tensor_scalar(
                    out=hb[:], in0=hb[:],
                    scalar1=mv[:, 0:1], scalar2=mv[:, 1:2],
                    op0=mybir.AluOpType.subtract, op1=mybir.AluOpType.mult,
                )
                nc.vector.tensor_mul(out=hb[:], in0=hb[:], in1=vec_t[:, 1, :])
                nc.vector.tensor_add(out=hb[:], in0=hb[:], in1=vec_t[:, 2, :])
                nc.sync.dma_start(out=out[half * 128:(half + 1) * 128, :],
                                  in_=hb[:])
    nc.finalize()
    return nc


_NC_CACHE = None


def _get_nc():
    global _NC_CACHE
    if _NC_CACHE is None:
        _NC_CACHE = _build_bass()
    return _NC_CACHE


def run(x, W, dense_w, dense_b, gamma, beta, trace=False):
    x = np.asarray(x, np.float32)
    wsb_np, dwp_np = _host_weights(np.asarray(W, np.float32),
                                   np.asarray(dense_w, np.float32))
    vecs_np = np.stack([
        np.asarray(dense_b, np.float32),
        np.asarray(gamma, np.float32),
        np.asarray(beta, np.float32),
    ])
    id4_np = _host_ident4()
    in_maps = []
    for c in range(NCORES):
        in_maps.append({
            "xt": _host_xt(x[c * BS:(c + 1) * BS]),
            "wsb": wsb_np,
            "dwp": dwp_np,
            "vecs": vecs_np,
            "ident4": id4_np,
        })
    res = run_bass_kernel_spmd(
        _get_nc(), in_maps, core_ids=list(range(NCORES)), trace=trace
    )
    out = np.concatenate([res.results[c]["out"] for c in range(NCORES)], axis=0)
    return out.astype(np.float32), res


def kernel(x, W, dense_w, dense_b, gamma, beta):
    out, _ = run(x, W, dense_w, dense_b, gamma, beta)
    return out


# revision 43
# speedup vs baseline: 1.0059x; 1.0059x over previous
"""Trainium2 Bass kernel for BilinearInteractionPlusLayer.

Math (per batch row b):
    pairs (i,j), i<j over F=40 fields, P=C(40,2)=780 pairs
    t[b,p,f] = sum_e x[b,i,e] * W[p,e,f]
    q[b,p]   = sum_f t[b,p,f] * x[b,j,f]
    h[b,d]   = sum_p q[b,p] * dense_w[p,d] + dense_b[d]
    out      = LayerNorm(h) * gamma + beta          (eps = 1e-3)

Sharding: data-parallel over batch, 2048 -> 256 rows on each of 8 cores.
W / dense_w / LN params are replicated. No collectives.

Per-core pipeline, pair math in a transposed "[feature x batch]" layout:
  - x arrives host-packed as bf16: xT[32*(j%4)+f, (j//4)*256 + b].
  - One "chunk" = (field i, j-group jg): stage-1 matmul
        lhsT = W chunk [32e x 128(c,f)] at row strip r = i%4
        rhs  = xT_i    [32e x 256b]
        out  = t chunk [128(c,f) x 256b] in its OWN PSUM bank
    Rounds of 4 chunks (strips 0..3) issue back-to-back; distinct row
    strips + distinct PSUM banks -> the 4 matmuls run concurrently in
    the PE array (row tiling).
  - Hadamard m = t * xT over three engine routes, balanced:
        A: ScalarE cast fp32->bf16, VectorE bf16 mul (2x mode)
        B: VectorE mul direct from PSUM (fp32 in, bf16 out)
        C: ScalarE cast, GpSimd bf16 mul
  - dw2 matmuls fold dense_w and the (pair,f) reduction:
        h4[32g+d, b] += dw2_k.T @ m_k     (g = row strip of chunk k)
    4-way column tiling -> 4 concurrent accumulating matmuls.
  - dw2 operand is built on-chip from a 27KB host tensor via 4
    partition-broadcast DMAs (instead of DMAing the 860KB expansion).
  - Tail: selector matmul + PE transpose + bn_stats LayerNorm.
"""

import itertools

import numpy as np

import concourse.bass as bass
from concourse import bacc, mybir
from concourse.bass_utils import run_bass_kernel_spmd
from concourse.tile import TileContext

F32 = mybir.dt.float32
BF16 = mybir.dt.bfloat16
NP_BF16 = mybir.dt.np(BF16)

B, F, E, P, D = 2048, 40, 32, 780, 16
NCORES = 8
BS = B // NCORES          # 256 batch rows per core
NJG = F // 4              # 10 j-groups of 4 fields
LN_EPS = 1e-3

# Hadamard route weights per PAIR-group of 2 chunks
# (A: cast+DVE 2x, B: DVE direct fp32-from-PSUM, C: cast+gpsimd)
ROUTE_W = (28, 42, 35)

# chunks moved to strip 3 to even the per-strip counts; each moved
# chunk's x_i is duplicated at partition strip 3 in an extra xT block.
# (empty: rebalancing measured net-neutral vs. scheduling noise)
MOVES = []
DUP_FIELDS = sorted({i for i, _ in MOVES})       # fields duplicated at strip 3
DUP_BLK = {i: NJG + m for m, i in enumerate(DUP_FIELDS)}
NXB = NJG + len(DUP_FIELDS)         # xT free blocks incl. duplicates


def _make_chunks():
    """One chunk = (field i, j-group jg): a [32e x 128(c,f)] stage-1 matmul.
    s = slot index within the chunk's row strip; xblk = xT free block
    holding x_i at partition strip r."""
    runs = [[] for _ in range(4)]    # per strip: list of (i, [jgs])
    for i in range(F):
        jgs = [jg for jg in range((i + 1) // 4, NJG) if (i, jg) not in MOVES]
        if jgs:
            runs[i % 4].append((i, jgs))
    chunks = []
    per_strip = [0, 0, 0, 0]
    for r in range(4):
        # even-length runs first, then odd: run boundaries land on even
        # slots so pair-groups almost never straddle two fields
        ordered = ([run for run in runs[r] if len(run[1]) % 2 == 0]
                   + [run for run in runs[r] if len(run[1]) % 2 == 1])
        for i, jgs in ordered:
            for jg in jgs:
                chunks.append(
                    {"i": i, "jg": jg, "r": r, "s": per_strip[r],
                     "k": len(chunks), "xblk": i // 4}
                )
                per_strip[r] += 1
    for mi, (i, jg) in enumerate(MOVES):
        chunks.append(
            {"i": i, "jg": jg, "r": 3, "s": per_strip[3],
             "k": len(chunks), "xblk": DUP_BLK[i]}
        )
        per_strip[3] += 1
    return chunks, per_strip


CHUNKS, PER_STRIP = _make_chunks()
NCH = len(CHUNKS)          # 210
SLOTS = max(PER_STRIP)     # 53 rounds
CH_BY_RS = {(c["r"], c["s"]): c for c in CHUNKS}


NPAIRS = sum((c + 1) // 2 for c in PER_STRIP)   # pair-groups of 2 slots


def _routes(n):
    """Per-group Hadamard route (0=A,1=B,2=C) via largest-remainder."""
    total = sum(ROUTE_W)
    nr = len(ROUTE_W)
    taken = [0] * nr
    out = []
    for k in range(n):
        deficits = [ROUTE_W[j] * (k + 1) / total - taken[j] for j in range(nr)]
        j = max(range(nr), key=lambda jj: deficits[jj])
        taken[j] += 1
        out.append(j)
    return out


ROUTES = _routes(NPAIRS)


def _host_weights(W, dense_w):
    """Pack W into per-strip stationary chunks and dense_w into dwpack
    [4, NCH*16]: dwpack[c, k*16+d] = dense_w[p(k,c), d] (zero if absent)."""
    pair_idx = {pq: n for n, pq in enumerate(itertools.combinations(range(F), 2))}
    wsb = np.zeros((128, SLOTS * 128), np.float32)
    dwp = np.zeros((4, NCH * 16), np.float32)
    for ch in CHUNKS:
        i, jg, r, s, k = ch["i"], ch["jg"], ch["r"], ch["s"], ch["k"]
        for c in range(4):
            j = 4 * jg + c
            if j <= i:
                continue
            p = pair_idx[(i, j)]
            wsb[32 * r:32 * r + 32, s * 128 + 32 * c:s * 128 + 32 * c + 32] = W[p]
            dwp[c, k * 16:(k + 1) * 16] = dense_w[p]
    return wsb.astype(NP_BF16), dwp.astype(NP_BF16)


def _host_xt(xc):
    """Per-core phase layout: xt[32*(j%4)+f, (j//4)*BS + b] = xc[b, j, f],
    plus duplicate blocks at strip 3 for the rebalanced chunks."""
    arr = xc.transpose(1, 2, 0)                    # [F, E, BS]
    arr = arr.reshape(NJG, 4, E, BS)               # [jg, c, f, b]
    arr = arr.transpose(1, 2, 0, 3)                # [c, f, jg, b]
    full = np.zeros((128, NXB * BS), np.float32)
    full[:, :NJG * BS] = arr.reshape(128, NJG * BS)
    for i, blk in DUP_BLK.items():
        full[96:128, blk * BS:(blk + 1) * BS] = xc[:, i, :].T
    return full.astype(NP_BF16)


def _host_ident4():
    """[128, 16] with a 16x16 identity at each 32-partition strip."""
    id4 = np.zeros((128, 16), np.float32)
    for g in range(4):
        id4[32 * g:32 * g + 16, :] = np.eye(16, dtype=np.float32)
    return id4


def _build_bass():
    nc = bacc.Bacc(trn_type="TRN2")
    xin = nc.dram_tensor("xt", [128, NXB * BS], BF16, kind="ExternalInput")
    wsb = nc.dram_tensor("wsb", [128, SLOTS * 128], BF16, kind="ExternalInput")
    dwp = nc.dram_tensor("dwp", [4, NCH * 16], BF16, kind="ExternalInput")
    vecs = nc.dram_tensor("vecs", [3, D], F32, kind="ExternalInput")
    id4 = nc.dram_tensor("ident4", [128, D], F32, kind="ExternalInput")
    out = nc.dram_tensor("out", [BS, D], F32, kind="ExternalOutput")

    with TileContext(nc) as tc:
        with (
            tc.tile_pool(name="const", bufs=1) as const,
            tc.tile_pool(name="cast", bufs=8) as cbuf,
            tc.tile_pool(name="mbuf", bufs=12) as mbuf,
            tc.tile_pool(name="tsegp", bufs=7, space="PSUM") as tsegp,
            tc.tile_pool(name="hpsp", bufs=1, space="PSUM") as hpsp,
            tc.tile_pool(name="lnp", bufs=2) as lnp,
        ):
            # ---- constants / inputs (first pieces sized so window 0 can
            # start as early as possible; spread across issue queues so
            # descriptor generation isn't serialized on one sequencer)
            xT = const.tile([128, NXB, BS], BF16)
            nc.sync.dma_start(out=xT[:], in_=xin[:, :])
            wsb_t = const.tile([128, SLOTS * 128], BF16)
            wcuts = [0, 6 * 128, 16 * 128, 30 * 128, SLOTS * 128]
            for a, b_ in zip(wcuts[:-1], wcuts[1:]):
                nc.sync.dma_start(out=wsb_t[:, a:b_], in_=wsb[:, a:b_])
            # dw2 expansion on-chip: dw2_t[32c+f', k*16+d] = dwp[c, k*16+d]
            # broadcast down 32 partitions per c-group
            dw2_t = const.tile([128, NCH * 16], BF16)
            for c in range(4):
                src = dwp[c:c + 1, :]
                nc.sync.dma_start(
                    out=dw2_t[32 * c:32 * c + 32, :],
                    in_=bass.AP(tensor=src.tensor, offset=src.offset,
                                ap=[[0, 32]] + [list(a) for a in src.ap[1:]]),
                )
            # vecs rows: 0 = dense_b, 1 = gamma, 2 = beta
            vec_t = const.tile([128, 3, D], F32)
            src = vecs[:, :]
            nc.sync.dma_start(
                out=vec_t[:],
                in_=bass.AP(tensor=src.tensor, offset=src.offset,
                            ap=[[0, 128]] + [list(a) for a in src.ap]),
            )
            id4_t = const.tile([128, D], F32)
            nc.sync.dma_start(out=id4_t[:], in_=id4[:, :])
            eps_t = const.tile([128, 1], F32)
            nc.vector.memset(eps_t[:], LN_EPS)

            # One PSUM bank holds h4 (4 col-tiled partial accumulators
            # [16d x 256b] at partition strips, free 0:256) and hsum
            # ([16d x 256b], free 256:512); ht reuses h4's range later.
            hcomb = hpsp.tile([128, 2 * BS], F32)
            h4 = hcomb[:, 0:BS]
            hsum = hcomb[0:D, BS:2 * BS]

            # ---- main pipeline over windows: window w = slots (2w, 2w+1)
            # of each strip.  Each (strip, window) owns a one-bank PSUM pair
            # tile so the 4 strips' stage-1 matmuls of a slot hit 4 distinct
            # banks and run concurrently (row tiling).
            def group_of(r, w):
                slots = [s for s in (2 * w, 2 * w + 1) if s < PER_STRIP[r]]
                return [CH_BY_RS[(r, s)] for s in slots]

            NW = (SLOTS + 1) // 2
            t_tiles = {}     # (r, w) -> psum pair tile
            m_tiles = {}     # (r, w) -> sbuf bf16 pair tile
            pair_idx = 0
            pair_route = {}  # (r, w) -> route

            def stage1(w):
                for u in range(2):
                    for r in range(4):
                        grp = group_of(r, w)
                        if u >= len(grp):
                            continue
                        if u == 0:
                            t_tiles[(r, w)] = tsegp.tile(
                                [128, 2, BS], F32, tag="t", name="tseg")
                        ch = grp[u]
                        s = ch["s"]
                        nc.tensor.matmul(
                            t_tiles[(r, w)][:, u, :],
                            lhsT=wsb_t[32 * r:32 * r + 32,
                                       s * 128:(s + 1) * 128],
                            rhs=xT[32 * r:32 * r + 32, ch["xblk"], :],
                            start=True, stop=True,
                            tile_position=(32 * r, 0),
                        )

            def hadamard(w):
                nonlocal pair_idx
                work = []
                for r in range(4):
                    grp = group_of(r, w)
                    if not grp:
                        continue
                    route = ROUTES[pair_idx]
                    pair_route[(r, w)] = route
                    pair_idx += 1
                    work.append((r, grp, route))
                for r, grp, route in work:
                    tt = t_tiles.pop((r, w))
                    m_t = mbuf.tile([128, 2, BS], BF16, tag="m")
                    m_tiles[(r, w)] = m_t
                    n = len(grp)
                    contig = n == 2 and grp[1]["jg"] == grp[0]["jg"] + 1
                    spans = ([(0, n, grp[0]["jg"])] if (contig or n == 1)
                             else [(0, 1, grp[0]["jg"]), (1, 1, grp[1]["jg"])])
                    for (u0, cnt, jg) in spans:
                        if route == 1:
                            nc.vector.tensor_mul(
                                out=m_t[:, u0:u0 + cnt, :],
                                in0=tt[:, u0:u0 + cnt, :],
                                in1=xT[:, jg:jg + cnt, :])
                        else:
                            tcast = cbuf.tile([128, 2, BS], BF16, tag="tc")
                            nc.scalar.copy(out=tcast[:, u0:u0 + cnt, :],
                                           in_=tt[:, u0:u0 + cnt, :])
                            eng = nc.vector if route == 0 else nc.gpsimd
                            eng.tensor_mul(
                                out=m_t[:, u0:u0 + cnt, :],
                                in0=tcast[:, u0:u0 + cnt, :],
                                in1=xT[:, jg:jg + cnt, :])

            def dw2_fold(w):
                for u in range(2):
                    for r in range(4):
                        grp = group_of(r, w)
                        if u >= len(grp):
                            continue
                        ch = grp[u]
                        k = ch["k"]
                        m_t = m_tiles[(r, w)]
                        nc.tensor.matmul(
                            h4[32 * r:32 * r + D, :],
                            lhsT=dw2_t[:, k * 16:(k + 1) * 16],
                            rhs=m_t[:, u, :],
                            start=(ch["s"] == 0),
                            stop=(ch["s"] == PER_STRIP[r] - 1),
                            tile_position=(0, 32 * r),
                        )
                for r in range(4):
                    m_tiles.pop((r, w), None)

            for w in range(NW + 2):
                if 0 <= w - 1 < NW:
                    hadamard(w - 1)
                if w < NW:
                    stage1(w)
                if 0 <= w - 2 < NW:
                    dw2_fold(w - 2)

            # ---- tail: combine the 4 partial h's with one selector matmul
            hg_sb = lnp.tile([128, BS], F32, tag="hgsb")
            nc.vector.memset(hg_sb[:], 0.0)
            for g in range(4):
                nc.scalar.copy(out=hg_sb[32 * g:32 * g + D, :],
                               in_=h4[32 * g:32 * g + D, :])
            nc.tensor.matmul(hsum, lhsT=id4_t[:], rhs=hg_sb[:],
                             start=True, stop=True)
            hsum_sb = lnp.tile([D, BS], F32, tag="hsum_sb")
            nc.scalar.copy(out=hsum_sb[:], in_=hsum)

            def ht_v(half):
                off = half * D
                return hcomb[0:128, off:off + D]

            for half in range(2):
                nc.tensor.transpose(
                    ht_v(half),
                    hsum_sb[:, half * 128:(half + 1) * 128],
                    id4_t[0:D, :],
                )
            # ---- LayerNorm per 128-row half
            for half in range(2):
                hb = lnp.tile([128, D], F32, tag="hb")
                nc.vector.tensor_add(out=hb[:], in0=ht_v(half),
                                     in1=vec_t[:, 0, :])
                stats = lnp.tile([128, 6], F32, tag="stats")
                nc.vector.bn_stats(out=stats[:], in_=hb[:])
                mv = lnp.tile([128, 2], F32, tag="mv")
                nc.vector.bn_aggr(out=mv[:], in_=stats[:])
                nc.scalar.activation(
                    out=mv[:, 1:2], in_=mv[:, 1:2],
                    func=mybir.ActivationFunctionType.Sqrt,
                    bias=eps_t[:], scale=1.0,
                )
                nc.vector.reciprocal(out=mv[:, 1:2], in_=mv[:, 1:2])
                nc.vector.tensor_scalar(
                    out=hb[:], in0=hb[:],
                    scalar1=mv[:, 0:1], scalar2=mv[:, 1:2],
                    op0=mybir.AluOpType.subtract, op1=mybir.AluOpType.mult,
                )
                nc.vector.tensor_mul(out=hb[:], in0=hb[:], in1=vec_t[:, 1, :])
                nc.vector.tensor_add(out=hb[:], in0=hb[:], in1=vec_t[:, 2, :])
                nc.sync.dma_start(out=out[half * 128:(half + 1) * 128, :],
                                  in_=hb[:])
    nc.finalize()
    return nc


_NC_CACHE = None


def _get_nc():
    global _NC_CACHE
    if _NC_CACHE is None:
        _NC_CACHE = _build_bass()
    return _NC_CACHE


def run(x, W, dense_w, dense_b, gamma, beta, trace=False):
    x = np.asarray(x, np.float32)
    wsb_np, dwp_np = _host_weights(np.asarray(W, np.float32),
                                   np.asarray(dense_w, np.float32))
    vecs_np = np.stack([
        np.asarray(dense_b, np.float32),
        np.asarray(gamma, np.float32),
        np.asarray(beta, np.float32),
    ])
    id4_np = _host_ident4()
    in_maps = []
    for c in range(NCORES):
        in_maps.append({
            "xt": _host_xt(x[c * BS:(c + 1) * BS]),
            "wsb": wsb_np,
            "dwp": dwp_np,
            "vecs": vecs_np,
            "ident4": id4_np,
        })
    res = run_bass_kernel_spmd(
        _get_nc(), in_maps, core_ids=list(range(NCORES)), trace=trace
    )
    out = np.concatenate([res.results[c]["out"] for c in range(NCORES)], axis=0)
    return out.astype(np.float32), res


def kernel(x, W, dense_w, dense_b, gamma, beta):
    out, _ = run(x, W, dense_w, dense_b, gamma, beta)
    return out


# revision 45
# speedup vs baseline: 1.0106x; 1.0046x over previous
"""Trainium2 Bass kernel for BilinearInteractionPlusLayer.

Math (per batch row b):
    pairs (i,j), i<j over F=40 fields, P=C(40,2)=780 pairs
    t[b,p,f] = sum_e x[b,i,e] * W[p,e,f]
    q[b,p]   = sum_f t[b,p,f] * x[b,j,f]
    h[b,d]   = sum_p q[b,p] * dense_w[p,d] + dense_b[d]
    out      = LayerNorm(h) * gamma + beta          (eps = 1e-3)

Sharding: data-parallel over batch, 2048 -> 256 rows on each of 8 cores.
W / dense_w / LN params are replicated. No collectives.

Per-core pipeline, pair math in a transposed "[feature x batch]" layout:
  - x arrives host-packed as bf16: xT[32*(j%4)+f, (j//4)*256 + b].
  - One "chunk" = (field i, j-group jg): stage-1 matmul
        lhsT = W chunk [32e x 128(c,f)] at row strip r = i%4
        rhs  = xT_i    [32e x 256b]
        out  = t chunk [128(c,f) x 256b] in its OWN PSUM bank
    Rounds of 4 chunks (strips 0..3) issue back-to-back; distinct row
    strips + distinct PSUM banks -> the 4 matmuls run concurrently in
    the PE array (row tiling).
  - Hadamard m = t * xT over three engine routes, balanced:
        A: ScalarE cast fp32->bf16, VectorE bf16 mul (2x mode)
        B: VectorE mul direct from PSUM (fp32 in, bf16 out)
        C: ScalarE cast, GpSimd bf16 mul
  - dw2 matmuls fold dense_w and the (pair,f) reduction:
        h4[32g+d, b] += dw2_k.T @ m_k     (g = row strip of chunk k)
    4-way column tiling -> 4 concurrent accumulating matmuls.
  - dw2 operand is built on-chip from a 27KB host tensor via 4
    partition-broadcast DMAs (instead of DMAing the 860KB expansion).
  - Tail: selector matmul + PE transpose + bn_stats LayerNorm.
"""

import itertools

import numpy as np

import concourse.bass as bass
from concourse import bacc, mybir
from concourse.bass_utils import run_bass_kernel_spmd
from concourse.tile import TileContext

F32 = mybir.dt.float32
BF16 = mybir.dt.bfloat16
NP_BF16 = mybir.dt.np(BF16)

B, F, E, P, D = 2048, 40, 32, 780, 16
NCORES = 8
BS = B // NCORES          # 256 batch rows per core
NJG = F // 4              # 10 j-groups of 4 fields
LN_EPS = 1e-3

# Hadamard route weights per PAIR-group of 2 chunks
# (A: cast+DVE 2x, B: DVE direct fp32-from-PSUM, C: cast+gpsimd)
ROUTE_W = (28, 42, 35)

# chunks moved to strip 3 to even the per-strip counts; each moved
# chunk's x_i is duplicated at partition strip 3 in an extra xT block.
# (empty: rebalancing measured net-neutral vs. scheduling noise)
MOVES = []
DUP_FIELDS = sorted({i for i, _ in MOVES})       # fields duplicated at strip 3
DUP_BLK = {i: NJG + m for m, i in enumerate(DUP_FIELDS)}
NXB = NJG + len(DUP_FIELDS)         # xT free blocks incl. duplicates


def _make_chunks():
    """One chunk = (field i, j-group jg): a [32e x 128(c,f)] stage-1 matmul.
    s = slot index within the chunk's row strip; xblk = xT free block
    holding x_i at partition strip r."""
    chunks = []
    per_strip = [0, 0, 0, 0]
    for i in range(F):
        for jg in range((i + 1) // 4, NJG):
            if (i, jg) in MOVES:
                r = 3
                xblk = DUP_BLK[i]
            else:
                r = i % 4
                xblk = i // 4
            chunks.append(
                {"i": i, "jg": jg, "r": r, "s": per_strip[r],
                 "k": len(chunks), "xblk": xblk}
            )
            per_strip[r] += 1
    return chunks, per_strip


CHUNKS, PER_STRIP = _make_chunks()
NCH = len(CHUNKS)          # 210
SLOTS = max(PER_STRIP)     # 53 rounds
CH_BY_RS = {(c["r"], c["s"]): c for c in CHUNKS}


NPAIRS = sum((c + 1) // 2 for c in PER_STRIP)   # pair-groups of 2 slots


def _routes(n):
    """Per-group Hadamard route (0=A,1=B,2=C) via largest-remainder."""
    total = sum(ROUTE_W)
    nr = len(ROUTE_W)
    taken = [0] * nr
    out = []
    for k in range(n):
        deficits = [ROUTE_W[j] * (k + 1) / total - taken[j] for j in range(nr)]
        j = max(range(nr), key=lambda jj: deficits[jj])
        taken[j] += 1
        out.append(j)
    return out


ROUTES = _routes(NPAIRS)


def _host_weights(W, dense_w):
    """Pack W into per-strip stationary chunks and dense_w into dwpack
    [4, NCH*16]: dwpack[c, k*16+d] = dense_w[p(k,c), d] (zero if absent)."""
    pair_idx = {pq: n for n, pq in enumerate(itertools.combinations(range(F), 2))}
    wsb = np.zeros((128, SLOTS * 128), np.float32)
    dwp = np.zeros((4, NCH * 16), np.float32)
    for ch in CHUNKS:
        i, jg, r, s, k = ch["i"], ch["jg"], ch["r"], ch["s"], ch["k"]
        for c in range(4):
            j = 4 * jg + c
            if j <= i:
                continue
            p = pair_idx[(i, j)]
            wsb[32 * r:32 * r + 32, s * 128 + 32 * c:s * 128 + 32 * c + 32] = W[p]
            dwp[c, k * 16:(k + 1) * 16] = dense_w[p]
    return wsb.astype(NP_BF16), dwp.astype(NP_BF16)


def _host_xt(xc):
    """Per-core phase layout: xt[32*(j%4)+f, (j//4)*BS + b] = xc[b, j, f],
    plus duplicate blocks at strip 3 for the rebalanced chunks."""
    arr = xc.transpose(1, 2, 0)                    # [F, E, BS]
    arr = arr.reshape(NJG, 4, E, BS)               # [jg, c, f, b]
    arr = arr.transpose(1, 2, 0, 3)                # [c, f, jg, b]
    full = np.zeros((128, NXB * BS), np.float32)
    full[:, :NJG * BS] = arr.reshape(128, NJG * BS)
    for i, blk in DUP_BLK.items():
        full[96:128, blk * BS:(blk + 1) * BS] = xc[:, i, :].T
    return full.astype(NP_BF16)


def _host_ident4():
    """[128, 16] with a 16x16 identity at each 32-partition strip."""
    id4 = np.zeros((128, 16), np.float32)
    for g in range(4):
        id4[32 * g:32 * g + 16, :] = np.eye(16, dtype=np.float32)
    return id4


def _build_bass():
    nc = bacc.Bacc(trn_type="TRN2")
    xin = nc.dram_tensor("xt", [128, NXB * BS], BF16, kind="ExternalInput")
    wsb = nc.dram_tensor("wsb", [128, SLOTS * 128], BF16, kind="ExternalInput")
    dwp = nc.dram_tensor("dwp", [4, NCH * 16], BF16, kind="ExternalInput")
    vecs = nc.dram_tensor("vecs", [3, D], F32, kind="ExternalInput")
    id4 = nc.dram_tensor("ident4", [128, D], F32, kind="ExternalInput")
    out = nc.dram_tensor("out", [BS, D], F32, kind="ExternalOutput")

    with TileContext(nc) as tc:
        with (
            tc.tile_pool(name="const", bufs=1) as const,
            tc.tile_pool(name="cast", bufs=8) as cbuf,
            tc.tile_pool(name="mbuf", bufs=12) as mbuf,
            tc.tile_pool(name="tsegp", bufs=7, space="PSUM") as tsegp,
            tc.tile_pool(name="hpsp", bufs=1, space="PSUM") as hpsp,
            tc.tile_pool(name="lnp", bufs=2) as lnp,
        ):
            # ---- constants / inputs (first pieces sized so window 0 can
            # start as early as possible; spread across issue queues so
            # descriptor generation isn't serialized on one sequencer)
            xT = const.tile([128, NXB, BS], BF16)
            nc.sync.dma_start(out=xT[:], in_=xin[:, :])
            wsb_t = const.tile([128, SLOTS * 128], BF16)
            wcuts = [0, 6 * 128, 16 * 128, 30 * 128, SLOTS * 128]
            for a, b_ in zip(wcuts[:-1], wcuts[1:]):
                nc.sync.dma_start(out=wsb_t[:, a:b_], in_=wsb[:, a:b_])
            # dw2 expansion on-chip: dw2_t[32c+f', k*16+d] = dwp[c, k*16+d]
            # broadcast down 32 partitions per c-group
            dw2_t = const.tile([128, NCH * 16], BF16)
            for c in range(4):
                src = dwp[c:c + 1, :]
                nc.sync.dma_start(
                    out=dw2_t[32 * c:32 * c + 32, :],
                    in_=bass.AP(tensor=src.tensor, offset=src.offset,
                                ap=[[0, 32]] + [list(a) for a in src.ap[1:]]),
                )
            # vecs rows: 0 = dense_b, 1 = gamma, 2 = beta
            vec_t = const.tile([128, 3, D], F32)
            src = vecs[:, :]
            nc.sync.dma_start(
                out=vec_t[:],
                in_=bass.AP(tensor=src.tensor, offset=src.offset,
                            ap=[[0, 128]] + [list(a) for a in src.ap]),
            )
            id4_t = const.tile([128, D], F32)
            nc.sync.dma_start(out=id4_t[:], in_=id4[:, :])
            eps_t = const.tile([128, 1], F32)
            nc.vector.memset(eps_t[:], LN_EPS)

            # One PSUM bank holds h4 (4 col-tiled partial accumulators
            # [16d x 256b] at partition strips, free 0:256) and hsum
            # ([16d x 256b], free 256:512); ht reuses h4's range later.
            hcomb = hpsp.tile([128, 2 * BS], F32)
            h4 = hcomb[:, 0:BS]
            hsum = hcomb[0:D, BS:2 * BS]

            # ---- main pipeline over windows: window w = slots (2w, 2w+1)
            # of each strip.  Each (strip, window) owns a one-bank PSUM pair
            # tile so the 4 strips' stage-1 matmuls of a slot hit 4 distinct
            # banks and run concurrently (row tiling).
            def group_of(r, w):
                slots = [s for s in (2 * w, 2 * w + 1) if s < PER_STRIP[r]]
                return [CH_BY_RS[(r, s)] for s in slots]

            NW = (SLOTS + 1) // 2
            t_tiles = {}     # (r, w) -> psum pair tile
            m_tiles = {}     # (r, w) -> sbuf bf16 pair tile
            pair_idx = 0
            pair_route = {}  # (r, w) -> route

            def stage1(w):
                for u in range(2):
                    for r in range(4):
                        grp = group_of(r, w)
                        if u >= len(grp):
                            continue
                        if u == 0:
                            t_tiles[(r, w)] = tsegp.tile(
                                [128, 2, BS], F32, tag="t", name="tseg")
                        ch = grp[u]
                        s = ch["s"]
                        nc.tensor.matmul(
                            t_tiles[(r, w)][:, u, :],
                            lhsT=wsb_t[32 * r:32 * r + 32,
                                       s * 128:(s + 1) * 128],
                            rhs=xT[32 * r:32 * r + 32, ch["xblk"], :],
                            start=True, stop=True,
                            tile_position=(32 * r, 0),
                        )

            def hadamard(w):
                nonlocal pair_idx
                work = []
                for r in range(4):
                    grp = group_of(r, w)
                    if not grp:
                        continue
                    route = ROUTES[pair_idx]
                    pair_route[(r, w)] = route
                    pair_idx += 1
                    work.append((r, grp, route))
                for r, grp, route in work:
                    tt = t_tiles.pop((r, w))
                    m_t = mbuf.tile([128, 2, BS], BF16, tag="m")
                    m_tiles[(r, w)] = m_t
                    n = len(grp)
                    contig = n == 2 and grp[1]["jg"] == grp[0]["jg"] + 1
                    spans = ([(0, n, grp[0]["jg"])] if (contig or n == 1)
                             else [(0, 1, grp[0]["jg"]), (1, 1, grp[1]["jg"])])
                    for (u0, cnt, jg) in spans:
                        if route == 1:
                            nc.vector.tensor_mul(
                                out=m_t[:, u0:u0 + cnt, :],
                                in0=tt[:, u0:u0 + cnt, :],
                                in1=xT[:, jg:jg + cnt, :])
                        else:
                            tcast = cbuf.tile([128, 2, BS], BF16, tag="tc")
                            nc.scalar.copy(out=tcast[:, u0:u0 + cnt, :],
                                           in_=tt[:, u0:u0 + cnt, :])
                            eng = nc.vector if route == 0 else nc.gpsimd
                            eng.tensor_mul(
                                out=m_t[:, u0:u0 + cnt, :],
                                in0=tcast[:, u0:u0 + cnt, :],
                                in1=xT[:, jg:jg + cnt, :])

            def dw2_fold(w):
                for u in range(2):
                    for r in range(4):
                        grp = group_of(r, w)
                        if u >= len(grp):
                            continue
                        ch = grp[u]
                        k = ch["k"]
                        m_t = m_tiles[(r, w)]
                        nc.tensor.matmul(
                            h4[32 * r:32 * r + D, :],
                            lhsT=dw2_t[:, k * 16:(k + 1) * 16],
                            rhs=m_t[:, u, :],
                            start=(ch["s"] == 0),
                            stop=(ch["s"] == PER_STRIP[r] - 1),
                            tile_position=(0, 32 * r),
                        )
                for r in range(4):
                    m_tiles.pop((r, w), None)

            for w in range(NW + 2):
                if w < NW:
                    stage1(w)
                if 0 <= w - 1 < NW:
                    hadamard(w - 1)
                if 0 <= w - 2 < NW:
                    dw2_fold(w - 2)

            # ---- tail: combine the 4 partial h's with one selector matmul
            hg_sb = lnp.tile([128, BS], F32, tag="hgsb")
            nc.vector.memset(hg_sb[:], 0.0)
            for g in range(4):
                nc.scalar.copy(out=hg_sb[32 * g:32 * g + D, :],
                               in_=h4[32 * g:32 * g + D, :])
            nc.tensor.matmul(hsum, lhsT=id4_t[:], rhs=hg_sb[:],
                             start=True, stop=True)
            hsum_sb = lnp.tile([D, BS], F32, tag="hsum_sb")
            nc.scalar.copy(out=hsum_sb[:], in_=hsum)

            def ht_v(half):
                off = half * D
                return hcomb[0:128, off:off + D]

            for half in range(2):
                nc.tensor.transpose(
                    ht_v(half),
                    hsum_sb[:, half * 128:(half + 1) * 128],
                    id4_t[0:D, :],
                )
            # ---- LayerNorm per 128-row half
            for half in range(2):
                hb = lnp.tile([128, D], F32, tag="hb")
                nc.vector.tensor_add(out=hb[:], in0=ht_v(half),
                                     in1=vec_t[:, 0, :])
                stats = lnp.tile([128, 6], F32, tag="stats")
                nc.vector.bn_stats(out=stats[:], in_=hb[:])
                mv = lnp.tile([128, 2], F32, tag="mv")
                nc.vector.bn_aggr(out=mv[:], in_=stats[:])
                nc.scalar.activation(
                    out=mv[:, 1:2], in_=mv[:, 1:2],
                    func=mybir.ActivationFunctionType.Sqrt,
                    bias=eps_t[:], scale=1.0,
                )
                nc.vector.reciprocal(out=mv[:, 1:2], in_=mv[:, 1:2])
                nc.vector.tensor_scalar(
                    out=hb[:], in0=hb[:],
                    scalar1=mv[:, 0:1], scalar2=mv[:, 1:2],
                    op0=mybir.AluOpType.subtract, op1=mybir.AluOpType.mult,
                )
                nc.vector.tensor_mul(out=hb[:], in0=hb[:], in1=vec_t[:, 1, :])
                nc.vector.tensor_add(out=hb[:], in0=hb[:], in1=vec_t[:, 2, :])
                nc.sync.dma_start(out=out[half * 128:(half + 1) * 128, :],
                                  in_=hb[:])
    nc.finalize()
    return nc


_NC_CACHE = None


def _get_nc():
    global _NC_CACHE
    if _NC_CACHE is None:
        _NC_CACHE = _build_bass()
    return _NC_CACHE


def run(x, W, dense_w, dense_b, gamma, beta, trace=False):
    x = np.asarray(x, np.float32)
    wsb_np, dwp_np = _host_weights(np.asarray(W, np.float32),
                                   np.asarray(dense_w, np.float32))
    vecs_np = np.stack([
        np.asarray(dense_b, np.float32),
        np.asarray(gamma, np.float32),
        np.asarray(beta, np.float32),
    ])
    id4_np = _host_ident4()
    in_maps = []
    for c in range(NCORES):
        in_maps.append({
            "xt": _host_xt(x[c * BS:(c + 1) * BS]),
            "wsb": wsb_np,
            "dwp": dwp_np,
            "vecs": vecs_np,
            "ident4": id4_np,
        })
    res = run_bass_kernel_spmd(
        _get_nc(), in_maps, core_ids=list(range(NCORES)), trace=trace
    )
    out = np.concatenate([res.results[c]["out"] for c in range(NCORES)], axis=0)
    return out.astype(np.float32), res


def kernel(x, W, dense_w, dense_b, gamma, beta):
    out, _ = run(x, W, dense_w, dense_b, gamma, beta)
    return out


# revision 46
# speedup vs baseline: 1.0158x; 1.0052x over previous
"""Trainium2 Bass kernel for BilinearInteractionPlusLayer.

Math (per batch row b):
    pairs (i,j), i<j over F=40 fields, P=C(40,2)=780 pairs
    t[b,p,f] = sum_e x[b,i,e] * W[p,e,f]
    q[b,p]   = sum_f t[b,p,f] * x[b,j,f]
    h[b,d]   = sum_p q[b,p] * dense_w[p,d] + dense_b[d]
    out      = LayerNorm(h) * gamma + beta          (eps = 1e-3)

Sharding: data-parallel over batch, 2048 -> 256 rows on each of 8 cores.
W / dense_w / LN params are replicated. No collectives.

Per-core pipeline, pair math in a transposed "[feature x batch]" layout:
  - x arrives host-packed as bf16: xT[32*(j%4)+f, (j//4)*256 + b].
  - One "chunk" = (field i, j-group jg): stage-1 matmul
        lhsT = W chunk [32e x 128(c,f)] at row strip r = i%4
        rhs  = xT_i    [32e x 256b]
        out  = t chunk [128(c,f) x 256b] in its OWN PSUM bank
    Rounds of 4 chunks (strips 0..3) issue back-to-back; distinct row
    strips + distinct PSUM banks -> the 4 matmuls run concurrently in
    the PE array (row tiling).
  - Hadamard m = t * xT over three engine routes, balanced:
        A: ScalarE cast fp32->bf16, VectorE bf16 mul (2x mode)
        B: VectorE mul direct from PSUM (fp32 in, bf16 out)
        C: ScalarE cast, GpSimd bf16 mul
  - dw2 matmuls fold dense_w and the (pair,f) reduction:
        h4[32g+d, b] += dw2_k.T @ m_k     (g = row strip of chunk k)
    4-way column tiling -> 4 concurrent accumulating matmuls.
  - dw2 operand is built on-chip from a 27KB host tensor via 4
    partition-broadcast DMAs (instead of DMAing the 860KB expansion).
  - Tail: selector matmul + PE transpose + bn_stats LayerNorm.
"""

import itertools

import numpy as np

import concourse.bass as bass
from concourse import bacc, mybir
from concourse.bass_utils import run_bass_kernel_spmd
from concourse.tile import TileContext

F32 = mybir.dt.float32
BF16 = mybir.dt.bfloat16
NP_BF16 = mybir.dt.np(BF16)

B, F, E, P, D = 2048, 40, 32, 780, 16
NCORES = 8
BS = B // NCORES          # 256 batch rows per core
NJG = F // 4              # 10 j-groups of 4 fields
LN_EPS = 1e-3

# Hadamard route weights per PAIR-group of 2 chunks
# (A: cast+DVE 2x, B: DVE direct fp32-from-PSUM, C: cast+gpsimd)
ROUTE_W = (29, 39, 37)

# chunks moved to strip 3 to even the per-strip counts; each moved
# chunk's x_i is duplicated at partition strip 3 in an extra xT block.
# (empty: rebalancing measured net-neutral vs. scheduling noise)
MOVES = []
DUP_FIELDS = sorted({i for i, _ in MOVES})       # fields duplicated at strip 3
DUP_BLK = {i: NJG + m for m, i in enumerate(DUP_FIELDS)}
NXB = NJG + len(DUP_FIELDS)         # xT free blocks incl. duplicates


def _make_chunks():
    """One chunk = (field i, j-group jg): a [32e x 128(c,f)] stage-1 matmul.
    s = slot index within the chunk's row strip; xblk = xT free block
    holding x_i at partition strip r."""
    chunks = []
    per_strip = [0, 0, 0, 0]
    for i in range(F):
        for jg in range((i + 1) // 4, NJG):
            if (i, jg) in MOVES:
                r = 3
                xblk = DUP_BLK[i]
            else:
                r = i % 4
                xblk = i // 4
            chunks.append(
                {"i": i, "jg": jg, "r": r, "s": per_strip[r],
                 "k": len(chunks), "xblk": xblk}
            )
            per_strip[r] += 1
    return chunks, per_strip


CHUNKS, PER_STRIP = _make_chunks()
NCH = len(CHUNKS)          # 210
SLOTS = max(PER_STRIP)     # 53 rounds
CH_BY_RS = {(c["r"], c["s"]): c for c in CHUNKS}


NPAIRS = sum((c + 1) // 2 for c in PER_STRIP)   # pair-groups of 2 slots


def _routes(n):
    """Per-group Hadamard route (0=A,1=B,2=C) via largest-remainder."""
    total = sum(ROUTE_W)
    nr = len(ROUTE_W)
    taken = [0] * nr
    out = []
    for k in range(n):
        deficits = [ROUTE_W[j] * (k + 1) / total - taken[j] for j in range(nr)]
        j = max(range(nr), key=lambda jj: deficits[jj])
        taken[j] += 1
        out.append(j)
    return out


ROUTES = _routes(NPAIRS)


def _host_weights(W, dense_w):
    """Pack W into per-strip stationary chunks and dense_w into dwpack
    [4, NCH*16]: dwpack[c, k*16+d] = dense_w[p(k,c), d] (zero if absent)."""
    pair_idx = {pq: n for n, pq in enumerate(itertools.combinations(range(F), 2))}
    wsb = np.zeros((128, SLOTS * 128), np.float32)
    dwp = np.zeros((4, NCH * 16), np.float32)
    for ch in CHUNKS:
        i, jg, r, s, k = ch["i"], ch["jg"], ch["r"], ch["s"], ch["k"]
        for c in range(4):
            j = 4 * jg + c
            if j <= i:
                continue
            p = pair_idx[(i, j)]
            wsb[32 * r:32 * r + 32, s * 128 + 32 * c:s * 128 + 32 * c + 32] = W[p]
            dwp[c, k * 16:(k + 1) * 16] = dense_w[p]
    return wsb.astype(NP_BF16), dwp.astype(NP_BF16)


def _host_xt(xc):
    """Per-core phase layout: xt[32*(j%4)+f, (j//4)*BS + b] = xc[b, j, f],
    plus duplicate blocks at strip 3 for the rebalanced chunks."""
    arr = xc.transpose(1, 2, 0)                    # [F, E, BS]
    arr = arr.reshape(NJG, 4, E, BS)               # [jg, c, f, b]
    arr = arr.transpose(1, 2, 0, 3)                # [c, f, jg, b]
    full = np.zeros((128, NXB * BS), np.float32)
    full[:, :NJG * BS] = arr.reshape(128, NJG * BS)
    for i, blk in DUP_BLK.items():
        full[96:128, blk * BS:(blk + 1) * BS] = xc[:, i, :].T
    return full.astype(NP_BF16)


def _host_ident4():
    """[128, 16] with a 16x16 identity at each 32-partition strip."""
    id4 = np.zeros((128, 16), np.float32)
    for g in range(4):
        id4[32 * g:32 * g + 16, :] = np.eye(16, dtype=np.float32)
    return id4


def _build_bass():
    nc = bacc.Bacc(trn_type="TRN2")
    xin = nc.dram_tensor("xt", [128, NXB * BS], BF16, kind="ExternalInput")
    wsb = nc.dram_tensor("wsb", [128, SLOTS * 128], BF16, kind="ExternalInput")
    dwp = nc.dram_tensor("dwp", [4, NCH * 16], BF16, kind="ExternalInput")
    vecs = nc.dram_tensor("vecs", [3, D], F32, kind="ExternalInput")
    id4 = nc.dram_tensor("ident4", [128, D], F32, kind="ExternalInput")
    out = nc.dram_tensor("out", [BS, D], F32, kind="ExternalOutput")

    with TileContext(nc) as tc:
        with (
            tc.tile_pool(name="const", bufs=1) as const,
            tc.tile_pool(name="cast", bufs=8) as cbuf,
            tc.tile_pool(name="mbuf", bufs=12) as mbuf,
            tc.tile_pool(name="tsegp", bufs=7, space="PSUM") as tsegp,
            tc.tile_pool(name="hpsp", bufs=1, space="PSUM") as hpsp,
            tc.tile_pool(name="lnp", bufs=2) as lnp,
        ):
            # ---- constants / inputs (first pieces sized so window 0 can
            # start as early as possible; spread across issue queues so
            # descriptor generation isn't serialized on one sequencer)
            xT = const.tile([128, NXB, BS], BF16)
            nc.sync.dma_start(out=xT[:], in_=xin[:, :])
            wsb_t = const.tile([128, SLOTS * 128], BF16)
            wcuts = [0, 6 * 128, 16 * 128, 30 * 128, SLOTS * 128]
            for a, b_ in zip(wcuts[:-1], wcuts[1:]):
                nc.sync.dma_start(out=wsb_t[:, a:b_], in_=wsb[:, a:b_])
            # dw2 expansion on-chip: dw2_t[32c+f', k*16+d] = dwp[c, k*16+d]
            # broadcast down 32 partitions per c-group
            dw2_t = const.tile([128, NCH * 16], BF16)
            for c in range(4):
                src = dwp[c:c + 1, :]
                nc.sync.dma_start(
                    out=dw2_t[32 * c:32 * c + 32, :],
                    in_=bass.AP(tensor=src.tensor, offset=src.offset,
                                ap=[[0, 32]] + [list(a) for a in src.ap[1:]]),
                )
            # vecs rows: 0 = dense_b, 1 = gamma, 2 = beta
            vec_t = const.tile([128, 3, D], F32)
            src = vecs[:, :]
            nc.sync.dma_start(
                out=vec_t[:],
                in_=bass.AP(tensor=src.tensor, offset=src.offset,
                            ap=[[0, 128]] + [list(a) for a in src.ap]),
            )
            id4_t = const.tile([128, D], F32)
            nc.sync.dma_start(out=id4_t[:], in_=id4[:, :])
            eps_t = const.tile([128, 1], F32)
            nc.vector.memset(eps_t[:], LN_EPS)

            # One PSUM bank holds h4 (4 col-tiled partial accumulators
            # [16d x 256b] at partition strips, free 0:256) and hsum
            # ([16d x 256b], free 256:512); ht reuses h4's range later.
            hcomb = hpsp.tile([128, 2 * BS], F32)
            h4 = hcomb[:, 0:BS]
            hsum = hcomb[0:D, BS:2 * BS]

            # ---- main pipeline over windows: window w = slots (2w, 2w+1)
            # of each strip.  Each (strip, window) owns a one-bank PSUM pair
            # tile so the 4 strips' stage-1 matmuls of a slot hit 4 distinct
            # banks and run concurrently (row tiling).
            def group_of(r, w):
                slots = [s for s in (2 * w, 2 * w + 1) if s < PER_STRIP[r]]
                return [CH_BY_RS[(r, s)] for s in slots]

            NW = (SLOTS + 1) // 2
            t_tiles = {}     # (r, w) -> psum pair tile
            m_tiles = {}     # (r, w) -> sbuf bf16 pair tile
            pair_idx = 0
            pair_route = {}  # (r, w) -> route

            def stage1(w):
                for u in range(2):
                    for r in range(4):
                        grp = group_of(r, w)
                        if u >= len(grp):
                            continue
                        if u == 0:
                            t_tiles[(r, w)] = tsegp.tile(
                                [128, 2, BS], F32, tag="t", name="tseg")
                        ch = grp[u]
                        s = ch["s"]
                        nc.tensor.matmul(
                            t_tiles[(r, w)][:, u, :],
                            lhsT=wsb_t[32 * r:32 * r + 32,
                                       s * 128:(s + 1) * 128],
                            rhs=xT[32 * r:32 * r + 32, ch["xblk"], :],
                            start=True, stop=True,
                            tile_position=(32 * r, 0),
                        )

            def hadamard(w):
                nonlocal pair_idx
                work = []
                for r in range(4):
                    grp = group_of(r, w)
                    if not grp:
                        continue
                    route = ROUTES[pair_idx]
                    pair_route[(r, w)] = route
                    pair_idx += 1
                    work.append((r, grp, route))
                for r, grp, route in work:
                    tt = t_tiles.pop((r, w))
                    m_t = mbuf.tile([128, 2, BS], BF16, tag="m")
                    m_tiles[(r, w)] = m_t
                    n = len(grp)
                    contig = n == 2 and grp[1]["jg"] == grp[0]["jg"] + 1
                    spans = ([(0, n, grp[0]["jg"])] if (contig or n == 1)
                             else [(0, 1, grp[0]["jg"]), (1, 1, grp[1]["jg"])])
                    for (u0, cnt, jg) in spans:
                        if route == 1:
                            nc.vector.tensor_mul(
                                out=m_t[:, u0:u0 + cnt, :],
                                in0=tt[:, u0:u0 + cnt, :],
                                in1=xT[:, jg:jg + cnt, :])
                        else:
                            tcast = cbuf.tile([128, 2, BS], BF16, tag="tc")
                            nc.scalar.copy(out=tcast[:, u0:u0 + cnt, :],
                                           in_=tt[:, u0:u0 + cnt, :])
                            eng = nc.vector if route == 0 else nc.gpsimd
                            eng.tensor_mul(
                                out=m_t[:, u0:u0 + cnt, :],
                                in0=tcast[:, u0:u0 + cnt, :],
                                in1=xT[:, jg:jg + cnt, :])

            def dw2_fold(w):
                for u in range(2):
                    for r in range(4):
                        grp = group_of(r, w)
                        if u >= len(grp):
                            continue
                        ch = grp[u]
                        k = ch["k"]
                        m_t = m_tiles[(r, w)]
                        nc.tensor.matmul(
                            h4[32 * r:32 * r + D, :],
                            lhsT=dw2_t[:, k * 16:(k + 1) * 16],
                            rhs=m_t[:, u, :],
                            start=(ch["s"] == 0),
                            stop=(ch["s"] == PER_STRIP[r] - 1),
                            tile_position=(0, 32 * r),
                        )
                for r in range(4):
                    m_tiles.pop((r, w), None)

            for w in range(NW + 2):
                if w < NW:
                    stage1(w)
                if 0 <= w - 1 < NW:
                    hadamard(w - 1)
                if 0 <= w - 2 < NW:
                    dw2_fold(w - 2)

            # ---- tail: combine the 4 partial h's with one selector matmul
            hg_sb = lnp.tile([128, BS], F32, tag="hgsb")
            nc.vector.memset(hg_sb[:], 0.0)
            for g in range(4):
                nc.scalar.copy(out=hg_sb[32 * g:32 * g + D, :],
                               in_=h4[32 * g:32 * g + D, :])
            nc.tensor.matmul(hsum, lhsT=id4_t[:], rhs=hg_sb[:],
                             start=True, stop=True)
            hsum_sb = lnp.tile([D, BS], F32, tag="hsum_sb")
            nc.scalar.copy(out=hsum_sb[:], in_=hsum)

            def ht_v(half):
                off = half * D
                return hcomb[0:128, off:off + D]

            for half in range(2):
                nc.tensor.transpose(
                    ht_v(half),
                    hsum_sb[:, half * 128:(half + 1) * 128],
                    id4_t[0:D, :],
                )
            # ---- LayerNorm per 128-row half
            for half in range(2):
                hb = lnp.tile([128, D], F32, tag="hb")
                nc.vector.tensor_add(out=hb[:], in0=ht_v(half),
                                     in1=vec_t[:, 0, :])
                stats = lnp.tile([128, 6], F32, tag="stats")
                nc.vector.bn_stats(out=stats[:], in_=hb[:])
                mv = lnp.tile([128, 2], F32, tag="mv")
                nc.vector.bn_aggr(out=mv[:], in_=stats[:])
                nc.scalar.activation(
                    out=mv[:, 1:2], in_=mv[:, 1:2],
                    func=mybir.ActivationFunctionType.Sqrt,
                    bias=eps_t[:], scale=1.0,
                )
                nc.vector.reciprocal(out=mv[:, 1:2], in_=mv[:, 1:2])
                nc.vector.tensor_scalar(
                    out=hb[:], in0=hb[:],
                    scalar1=mv[:, 0:1], scalar2=mv[:, 1:2],
                    op0=mybir.AluOpType.subtract, op1=mybir.AluOpType.mult,
                )
                nc.vector.tensor_mul(out=hb[:], in0=hb[:], in1=vec_t[:, 1, :])
                nc.vector.tensor_add(out=hb[:], in0=hb[:], in1=vec_t[:, 2, :])
                nc.sync.dma_start(out=out[half * 128:(half + 1) * 128, :],
                                  in_=hb[:])
    nc.finalize()
    return nc


_NC_CACHE = None


def _get_nc():
    global _NC_CACHE
    if _NC_CACHE is None:
        _NC_CACHE = _build_bass()
    return _NC_CACHE


def run(x, W, dense_w, dense_b, gamma, beta, trace=False):
    x = np.asarray(x, np.float32)
    wsb_np, dwp_np = _host_weights(np.asarray(W, np.float32),
                                   np.asarray(dense_w, np.float32))
    vecs_np = np.stack([
        np.asarray(dense_b, np.float32),
        np.asarray(gamma, np.float32),
        np.asarray(beta, np.float32),
    ])
    id4_np = _host_ident4()
    in_maps = []
    for c in range(NCORES):
        in_maps.append({
            "xt": _host_xt(x[c * BS:(c + 1) * BS]),
            "wsb": wsb_np,
            "dwp": dwp_np,
            "vecs": vecs_np,
            "ident4": id4_np,
        })
    res = run_bass_kernel_spmd(
        _get_nc(), in_maps, core_ids=list(range(NCORES)), trace=trace
    )
    out = np.concatenate([res.results[c]["out"] for c in range(NCORES)], axis=0)
    return out.astype(np.float32), res


def kernel(x, W, dense_w, dense_b, gamma, beta):
    out, _ = run(x, W, dense_w, dense_b, gamma, beta)
    return out


# revision 49
# speedup vs baseline: 1.0407x; 1.0245x over previous
"""Trainium2 Bass kernel for BilinearInteractionPlusLayer.

Math (per batch row b):
    pairs (i,j), i<j over F=40 fields, P=C(40,2)=780 pairs
    t[b,p,f] = sum_e x[b,i,e] * W[p,e,f]
    q[b,p]   = sum_f t[b,p,f] * x[b,j,f]
    h[b,d]   = sum_p q[b,p] * dense_w[p,d] + dense_b[d]
    out      = LayerNorm(h) * gamma + beta          (eps = 1e-3)

Sharding: data-parallel over batch, 2048 -> 256 rows on each of 8 cores.
W / dense_w / LN params are replicated. No collectives.

Per-core pipeline, pair math in a transposed "[feature x batch]" layout:
  - x arrives host-packed as bf16: xT[32*(j%4)+f, (j//4)*256 + b].
  - One "chunk" = (field i, j-group jg): stage-1 matmul
        lhsT = W chunk [32e x 128(c,f)] at row strip r = i%4
        rhs  = xT_i    [32e x 256b]
        out  = t chunk [128(c,f) x 256b] in its OWN PSUM bank
    Rounds of 4 chunks (strips 0..3) issue back-to-back; distinct row
    strips + distinct PSUM banks -> the 4 matmuls run concurrently in
    the PE array (row tiling).
  - Hadamard m = t * xT over three engine routes, balanced:
        A: ScalarE cast fp32->bf16, VectorE bf16 mul (2x mode)
        B: VectorE mul direct from PSUM (fp32 in, bf16 out)
        C: ScalarE cast, GpSimd bf16 mul
  - dw2 matmuls fold dense_w and the (pair,f) reduction:
        h4[32g+d, b] += dw2_k.T @ m_k     (g = row strip of chunk k)
    4-way column tiling -> 4 concurrent accumulating matmuls.
  - dw2 operand is built on-chip from a 27KB host tensor via 4
    partition-broadcast DMAs (instead of DMAing the 860KB expansion).
  - Tail: selector matmul + PE transpose + bn_stats LayerNorm.
"""

import itertools

import numpy as np

import concourse.bass as bass
from concourse import bacc, mybir
from concourse.bass_utils import run_bass_kernel_spmd
from concourse.tile import TileContext

F32 = mybir.dt.float32
BF16 = mybir.dt.bfloat16
NP_BF16 = mybir.dt.np(BF16)

B, F, E, P, D = 2048, 40, 32, 780, 16
NCORES = 8
BS = B // NCORES          # 256 batch rows per core
NJG = F // 4              # 10 j-groups of 4 fields
LN_EPS = 1e-3

# Hadamard route weights per PAIR-group of 2 chunks
# (A: cast+DVE 2x, B: DVE direct fp32-from-PSUM, C: cast+gpsimd)
ROUTE_W = (29, 39, 37)

# chunks moved to strip 3 to even the per-strip counts; each moved
# chunk's x_i is duplicated at partition strip 3 in an extra xT block.
# (empty: rebalancing measured net-neutral vs. scheduling noise)
MOVES = []
DUP_FIELDS = sorted({i for i, _ in MOVES})       # fields duplicated at strip 3
DUP_BLK = {i: NJG + m for m, i in enumerate(DUP_FIELDS)}
NXB = NJG + len(DUP_FIELDS)         # xT free blocks incl. duplicates


def _make_chunks():
    """One chunk = (field i, j-group jg): a [32e x 128(c,f)] stage-1 matmul.
    s = slot index within the chunk's row strip; xblk = xT free block
    holding x_i at partition strip r."""
    chunks = []
    per_strip = [0, 0, 0, 0]
    for i in range(F):
        for jg in range((i + 1) // 4, NJG):
            if (i, jg) in MOVES:
                r = 3
                xblk = DUP_BLK[i]
            else:
                r = i % 4
                xblk = i // 4
            chunks.append(
                {"i": i, "jg": jg, "r": r, "s": per_strip[r],
                 "k": len(chunks), "xblk": xblk}
            )
            per_strip[r] += 1
    return chunks, per_strip


CHUNKS, PER_STRIP = _make_chunks()
NCH = len(CHUNKS)          # 210
SLOTS = max(PER_STRIP)     # 53 rounds
CH_BY_RS = {(c["r"], c["s"]): c for c in CHUNKS}


NPAIRS = sum((c + 1) // 2 for c in PER_STRIP)   # pair-groups of 2 slots


def _routes(n):
    """Per-group Hadamard route (0=A,1=B,2=C) via largest-remainder."""
    total = sum(ROUTE_W)
    nr = len(ROUTE_W)
    taken = [0] * nr
    out = []
    for k in range(n):
        deficits = [ROUTE_W[j] * (k + 1) / total - taken[j] for j in range(nr)]
        j = max(range(nr), key=lambda jj: deficits[jj])
        taken[j] += 1
        out.append(j)
    return out


ROUTES = _routes(NPAIRS)


def _host_weights(W, dense_w):
    """Pack W into per-strip stationary chunks and dense_w into dwpack
    [4, NCH*16]: dwpack[c, k*16+d] = dense_w[p(k,c), d] (zero if absent)."""
    pair_idx = {pq: n for n, pq in enumerate(itertools.combinations(range(F), 2))}
    wsb = np.zeros((128, SLOTS * 128), np.float32)
    dwp = np.zeros((4, NCH * 16), np.float32)
    for ch in CHUNKS:
        i, jg, r, s, k = ch["i"], ch["jg"], ch["r"], ch["s"], ch["k"]
        for c in range(4):
            j = 4 * jg + c
            if j <= i:
                continue
            p = pair_idx[(i, j)]
            wsb[32 * r:32 * r + 32, s * 128 + 32 * c:s * 128 + 32 * c + 32] = W[p]
            dwp[c, k * 16:(k + 1) * 16] = dense_w[p]
    return wsb.astype(NP_BF16), dwp.astype(NP_BF16)


def _host_xt(xc):
    """Per-core phase layout: xt[32*(j%4)+f, (j//4)*BS + b] = xc[b, j, f],
    plus duplicate blocks at strip 3 for the rebalanced chunks."""
    arr = xc.transpose(1, 2, 0)                    # [F, E, BS]
    arr = arr.reshape(NJG, 4, E, BS)               # [jg, c, f, b]
    arr = arr.transpose(1, 2, 0, 3)                # [c, f, jg, b]
    full = np.zeros((128, NXB * BS), np.float32)
    full[:, :NJG * BS] = arr.reshape(128, NJG * BS)
    for i, blk in DUP_BLK.items():
        full[96:128, blk * BS:(blk + 1) * BS] = xc[:, i, :].T
    return full.astype(NP_BF16)


def _host_ident4():
    """[128, 16] with a 16x16 identity at each 32-partition strip."""
    id4 = np.zeros((128, 16), np.float32)
    for g in range(4):
        id4[32 * g:32 * g + 16, :] = np.eye(16, dtype=np.float32)
    return id4


def _build_bass():
    nc = bacc.Bacc(trn_type="TRN2")
    xin = nc.dram_tensor("xt", [128, NXB * BS], BF16, kind="ExternalInput")
    wsb = nc.dram_tensor("wsb", [128, SLOTS * 128], BF16, kind="ExternalInput")
    dwp = nc.dram_tensor("dwp", [4, NCH * 16], BF16, kind="ExternalInput")
    vecs = nc.dram_tensor("vecs", [3, D], F32, kind="ExternalInput")
    id4 = nc.dram_tensor("ident4", [128, D], F32, kind="ExternalInput")
    out = nc.dram_tensor("out", [BS, D], F32, kind="ExternalOutput")

    with TileContext(nc) as tc:
        with (
            tc.tile_pool(name="const", bufs=1) as const,
            tc.tile_pool(name="cast", bufs=8) as cbuf,
            tc.tile_pool(name="mbuf", bufs=12) as mbuf,
            tc.tile_pool(name="tsegp", bufs=7, space="PSUM") as tsegp,
            tc.tile_pool(name="hpsp", bufs=1, space="PSUM") as hpsp,
            tc.tile_pool(name="lnp", bufs=2) as lnp,
        ):
            # ---- constants / inputs (first pieces sized so window 0 can
            # start as early as possible; spread across issue queues so
            # descriptor generation isn't serialized on one sequencer)
            xT = const.tile([128, NXB, BS], BF16)
            nc.sync.dma_start(out=xT[:], in_=xin[:, :])
            wsb_t = const.tile([128, SLOTS * 128], BF16)
            wcuts = [0, 6 * 128, 16 * 128, 30 * 128, SLOTS * 128]
            for a, b_ in zip(wcuts[:-1], wcuts[1:]):
                nc.sync.dma_start(out=wsb_t[:, a:b_], in_=wsb[:, a:b_])
            # dw2 expansion on-chip: dw2_t[32c+f', k*16+d] = dwp[c, k*16+d]
            # broadcast down 32 partitions per c-group
            dw2_t = const.tile([128, NCH * 16], BF16)
            for c in range(4):
                src = dwp[c:c + 1, :]
                nc.sync.dma_start(
                    out=dw2_t[32 * c:32 * c + 32, :],
                    in_=bass.AP(tensor=src.tensor, offset=src.offset,
                                ap=[[0, 32]] + [list(a) for a in src.ap[1:]]),
                )
            # vecs rows: 0 = dense_b, 1 = gamma, 2 = beta
            vec_t = const.tile([128, 3, D], F32)
            src = vecs[:, :]
            nc.sync.dma_start(
                out=vec_t[:],
                in_=bass.AP(tensor=src.tensor, offset=src.offset,
                            ap=[[0, 128]] + [list(a) for a in src.ap]),
            )
            id4_t = const.tile([128, D], F32)
            nc.sync.dma_start(out=id4_t[:], in_=id4[:, :])
            eps_t = const.tile([128, 1], F32)
            nc.vector.memset(eps_t[:], LN_EPS)

            # One PSUM bank holds h4 (4 col-tiled partial accumulators
            # [16d x 256b] at partition strips, free 0:256) and hsum
            # ([16d x 256b], free 256:512); ht reuses h4's range later.
            hcomb = hpsp.tile([128, 2 * BS], F32)
            h4 = hcomb[:, 0:BS]
            hsum = hcomb[0:D, BS:2 * BS]
            # zero h4's bank up front: rows outside the four 16-row
            # accumulator strips are never written by dw2 matmuls, and the
            # tail reads the full tile in one copy
            nc.vector.memset(h4[:], 0.0)

            # ---- main pipeline over windows: window w = slots (2w, 2w+1)
            # of each strip.  Each (strip, window) owns a one-bank PSUM pair
            # tile so the 4 strips' stage-1 matmuls of a slot hit 4 distinct
            # banks and run concurrently (row tiling).
            def group_of(r, w):
                slots = [s for s in (2 * w, 2 * w + 1) if s < PER_STRIP[r]]
                return [CH_BY_RS[(r, s)] for s in slots]

            NW = (SLOTS + 1) // 2
            t_tiles = {}     # (r, w) -> psum pair tile
            m_tiles = {}     # (r, w) -> sbuf bf16 pair tile
            pair_idx = 0
            pair_route = {}  # (r, w) -> route

            def stage1(w):
                for u in range(2):
                    for r in range(4):
                        grp = group_of(r, w)
                        if u >= len(grp):
                            continue
                        if u == 0:
                            t_tiles[(r, w)] = tsegp.tile(
                                [128, 2, BS], F32, tag="t", name="tseg")
                        ch = grp[u]
                        s = ch["s"]
                        nc.tensor.matmul(
                            t_tiles[(r, w)][:, u, :],
                            lhsT=wsb_t[32 * r:32 * r + 32,
                                       s * 128:(s + 1) * 128],
                            rhs=xT[32 * r:32 * r + 32, ch["xblk"], :],
                            start=True, stop=True,
                            tile_position=(32 * r, 0),
                        )

            def hadamard(w):
                nonlocal pair_idx
                work = []
                for r in range(4):
                    grp = group_of(r, w)
                    if not grp:
                        continue
                    route = ROUTES[pair_idx]
                    if w >= NW - 2:
                        # drain path: single-op route has the lowest latency
                        route = 1
                    pair_route[(r, w)] = route
                    pair_idx += 1
                    work.append((r, grp, route))
                for r, grp, route in work:
                    tt = t_tiles.pop((r, w))
                    m_t = mbuf.tile([128, 2, BS], BF16, tag="m")
                    m_tiles[(r, w)] = m_t
                    n = len(grp)
                    contig = n == 2 and grp[1]["jg"] == grp[0]["jg"] + 1
                    spans = ([(0, n, grp[0]["jg"])] if (contig or n == 1)
                             else [(0, 1, grp[0]["jg"]), (1, 1, grp[1]["jg"])])
                    for (u0, cnt, jg) in spans:
                        if route == 1:
                            nc.vector.tensor_mul(
                                out=m_t[:, u0:u0 + cnt, :],
                                in0=tt[:, u0:u0 + cnt, :],
                                in1=xT[:, jg:jg + cnt, :])
                        else:
                            tcast = cbuf.tile([128, 2, BS], BF16, tag="tc")
                            nc.scalar.copy(out=tcast[:, u0:u0 + cnt, :],
                                           in_=tt[:, u0:u0 + cnt, :])
                            eng = nc.vector if route == 0 else nc.gpsimd
                            eng.tensor_mul(
                                out=m_t[:, u0:u0 + cnt, :],
                                in0=tcast[:, u0:u0 + cnt, :],
                                in1=xT[:, jg:jg + cnt, :])

            def dw2_fold(w):
                for u in range(2):
                    for r in range(4):
                        grp = group_of(r, w)
                        if u >= len(grp):
                            continue
                        ch = grp[u]
                        k = ch["k"]
                        m_t = m_tiles[(r, w)]
                        nc.tensor.matmul(
                            h4[32 * r:32 * r + D, :],
                            lhsT=dw2_t[:, k * 16:(k + 1) * 16],
                            rhs=m_t[:, u, :],
                            start=(ch["s"] == 0),
                            stop=(ch["s"] == PER_STRIP[r] - 1),
                            tile_position=(0, 32 * r),
                        )
                for r in range(4):
                    m_tiles.pop((r, w), None)

            for w in range(NW + 2):
                if w < NW:
                    stage1(w)
                if 0 <= w - 1 < NW:
                    hadamard(w - 1)
                if 0 <= w - 2 < NW:
                    dw2_fold(w - 2)

            # ---- tail: combine the 4 partial h's with one selector matmul
            # (one full-tile copy; id4's zero rows mask the unused strips)
            hg_sb = lnp.tile([128, BS], F32, tag="hgsb")
            nc.vector.tensor_copy(out=hg_sb[:], in_=h4[:])
            nc.tensor.matmul(hsum, lhsT=id4_t[:], rhs=hg_sb[:],
                             start=True, stop=True)
            hsum_sb = lnp.tile([D, BS], F32, tag="hsum_sb")
            nc.scalar.copy(out=hsum_sb[:], in_=hsum)

            def ht_v(half):
                off = half * D
                return hcomb[0:128, off:off + D]

            for half in range(2):
                nc.tensor.transpose(
                    ht_v(half),
                    hsum_sb[:, half * 128:(half + 1) * 128],
                    id4_t[0:D, :],
                )
            # ---- LayerNorm per 128-row half
            for half in range(2):
                hb = lnp.tile([128, D], F32, tag="hb")
                nc.vector.tensor_add(out=hb[:], in0=ht_v(half),
                                     in1=vec_t[:, 0, :])
                stats = lnp.tile([128, 6], F32, tag="stats")
                nc.vector.bn_stats(out=stats[:], in_=hb[:])
                mv = lnp.tile([128, 2], F32, tag="mv")
                nc.vector.bn_aggr(out=mv[:], in_=stats[:])
                nc.scalar.activation(
                    out=mv[:, 1:2], in_=mv[:, 1:2],
                    func=mybir.ActivationFunctionType.Sqrt,
                    bias=eps_t[:], scale=1.0,
                )
                nc.vector.reciprocal(out=mv[:, 1:2], in_=mv[:, 1:2])
                nc.vector.tensor_scalar(
                    out=hb[:], in0=hb[:],
                    scalar1=mv[:, 0:1], scalar2=mv[:, 1:2],
                    op0=mybir.AluOpType.subtract, op1=mybir.AluOpType.mult,
                )
                nc.vector.tensor_mul(out=hb[:], in0=hb[:], in1=vec_t[:, 1, :])
                nc.vector.tensor_add(out=hb[:], in0=hb[:], in1=vec_t[:, 2, :])
                nc.sync.dma_start(out=out[half * 128:(half + 1) * 128, :],
                                  in_=hb[:])
    nc.finalize()
    return nc


_NC_CACHE = None


def _get_nc():
    global _NC_CACHE
    if _NC_CACHE is None:
        _NC_CACHE = _build_bass()
    return _NC_CACHE


def run(x, W, dense_w, dense_b, gamma, beta, trace=False):
    x = np.asarray(x, np.float32)
    wsb_np, dwp_np = _host_weights(np.asarray(W, np.float32),
                                   np.asarray(dense_w, np.float32))
    vecs_np = np.stack([
        np.asarray(dense_b, np.float32),
        np.asarray(gamma, np.float32),
        np.asarray(beta, np.float32),
    ])
    id4_np = _host_ident4()
    in_maps = []
    for c in range(NCORES):
        in_maps.append({
            "xt": _host_xt(x[c * BS:(c + 1) * BS]),
            "wsb": wsb_np,
            "dwp": dwp_np,
            "vecs": vecs_np,
            "ident4": id4_np,
        })
    res = run_bass_kernel_spmd(
        _get_nc(), in_maps, core_ids=list(range(NCORES)), trace=trace
    )
    out = np.concatenate([res.results[c]["out"] for c in range(NCORES)], axis=0)
    return out.astype(np.float32), res


def kernel(x, W, dense_w, dense_b, gamma, beta):
    out, _ = run(x, W, dense_w, dense_b, gamma, beta)
    return out


# revision 50
# speedup vs baseline: 1.0466x; 1.0057x over previous
"""Trainium2 Bass kernel for BilinearInteractionPlusLayer.

Math (per batch row b):
    pairs (i,j), i<j over F=40 fields, P=C(40,2)=780 pairs
    t[b,p,f] = sum_e x[b,i,e] * W[p,e,f]
    q[b,p]   = sum_f t[b,p,f] * x[b,j,f]
    h[b,d]   = sum_p q[b,p] * dense_w[p,d] + dense_b[d]
    out      = LayerNorm(h) * gamma + beta          (eps = 1e-3)

Sharding: data-parallel over batch, 2048 -> 256 rows on each of 8 cores.
W / dense_w / LN params are replicated. No collectives.

Per-core pipeline, pair math in a transposed "[feature x batch]" layout:
  - x arrives host-packed as bf16: xT[32*(j%4)+f, (j//4)*256 + b].
  - One "chunk" = (field i, j-group jg): stage-1 matmul
        lhsT = W chunk [32e x 128(c,f)] at row strip r = i%4
        rhs  = xT_i    [32e x 256b]
        out  = t chunk [128(c,f) x 256b] in its OWN PSUM bank
    Rounds of 4 chunks (strips 0..3) issue back-to-back; distinct row
    strips + distinct PSUM banks -> the 4 matmuls run concurrently in
    the PE array (row tiling).
  - Hadamard m = t * xT over three engine routes, balanced:
        A: ScalarE cast fp32->bf16, VectorE bf16 mul (2x mode)
        B: VectorE mul direct from PSUM (fp32 in, bf16 out)
        C: ScalarE cast, GpSimd bf16 mul
  - dw2 matmuls fold dense_w and the (pair,f) reduction:
        h4[32g+d, b] += dw2_k.T @ m_k     (g = row strip of chunk k)
    4-way column tiling -> 4 concurrent accumulating matmuls.
  - dw2 operand is built on-chip from a 27KB host tensor via 4
    partition-broadcast DMAs (instead of DMAing the 860KB expansion).
  - Tail: selector matmul + PE transpose + bn_stats LayerNorm.
"""

import itertools

import numpy as np

import concourse.bass as bass
from concourse import bacc, mybir
from concourse.bass_utils import run_bass_kernel_spmd
from concourse.tile import TileContext

F32 = mybir.dt.float32
BF16 = mybir.dt.bfloat16
NP_BF16 = mybir.dt.np(BF16)

B, F, E, P, D = 2048, 40, 32, 780, 16
NCORES = 8
BS = B // NCORES          # 256 batch rows per core
NJG = F // 4              # 10 j-groups of 4 fields
LN_EPS = 1e-3

# Hadamard route weights per PAIR-group of 2 chunks
# (A: cast+DVE 2x, B: DVE direct fp32-from-PSUM, C: cast+gpsimd)
ROUTE_W = (29, 39, 37)

# chunks moved to strip 3 to even the per-strip counts; each moved
# chunk's x_i is duplicated at partition strip 3 in an extra xT block.
# (empty: rebalancing measured net-neutral vs. scheduling noise)
MOVES = []
DUP_FIELDS = sorted({i for i, _ in MOVES})       # fields duplicated at strip 3
DUP_BLK = {i: NJG + m for m, i in enumerate(DUP_FIELDS)}
NXB = NJG + len(DUP_FIELDS)         # xT free blocks incl. duplicates


def _make_chunks():
    """One chunk = (field i, j-group jg): a [32e x 128(c,f)] stage-1 matmul.
    s = slot index within the chunk's row strip; xblk = xT free block
    holding x_i at partition strip r."""
    chunks = []
    per_strip = [0, 0, 0, 0]
    for i in range(F):
        for jg in range((i + 1) // 4, NJG):
            if (i, jg) in MOVES:
                r = 3
                xblk = DUP_BLK[i]
            else:
                r = i % 4
                xblk = i // 4
            chunks.append(
                {"i": i, "jg": jg, "r": r, "s": per_strip[r],
                 "k": len(chunks), "xblk": xblk}
            )
            per_strip[r] += 1
    return chunks, per_strip


CHUNKS, PER_STRIP = _make_chunks()
NCH = len(CHUNKS)          # 210
SLOTS = max(PER_STRIP)     # 53 rounds
CH_BY_RS = {(c["r"], c["s"]): c for c in CHUNKS}


NPAIRS = sum((c + 1) // 2 for c in PER_STRIP)   # pair-groups of 2 slots


def _routes(n):
    """Per-group Hadamard route (0=A,1=B,2=C) via largest-remainder."""
    total = sum(ROUTE_W)
    nr = len(ROUTE_W)
    taken = [0] * nr
    out = []
    for k in range(n):
        deficits = [ROUTE_W[j] * (k + 1) / total - taken[j] for j in range(nr)]
        j = max(range(nr), key=lambda jj: deficits[jj])
        taken[j] += 1
        out.append(j)
    return out


ROUTES = _routes(NPAIRS)


def _host_weights(W, dense_w):
    """Pack W into per-strip stationary chunks and dense_w into dwpack
    [4, NCH*16]: dwpack[c, k*16+d] = dense_w[p(k,c), d] (zero if absent)."""
    pair_idx = {pq: n for n, pq in enumerate(itertools.combinations(range(F), 2))}
    wsb = np.zeros((128, SLOTS * 128), np.float32)
    dwp = np.zeros((4, NCH * 16), np.float32)
    for ch in CHUNKS:
        i, jg, r, s, k = ch["i"], ch["jg"], ch["r"], ch["s"], ch["k"]
        for c in range(4):
            j = 4 * jg + c
            if j <= i:
                continue
            p = pair_idx[(i, j)]
            wsb[32 * r:32 * r + 32, s * 128 + 32 * c:s * 128 + 32 * c + 32] = W[p]
            dwp[c, k * 16:(k + 1) * 16] = dense_w[p]
    return wsb.astype(NP_BF16), dwp.astype(NP_BF16)


def _host_xt(xc):
    """Per-core phase layout: xt[32*(j%4)+f, (j//4)*BS + b] = xc[b, j, f],
    plus duplicate blocks at strip 3 for the rebalanced chunks."""
    arr = xc.transpose(1, 2, 0)                    # [F, E, BS]
    arr = arr.reshape(NJG, 4, E, BS)               # [jg, c, f, b]
    arr = arr.transpose(1, 2, 0, 3)                # [c, f, jg, b]
    full = np.zeros((128, NXB * BS), np.float32)
    full[:, :NJG * BS] = arr.reshape(128, NJG * BS)
    for i, blk in DUP_BLK.items():
        full[96:128, blk * BS:(blk + 1) * BS] = xc[:, i, :].T
    return full.astype(NP_BF16)


def _host_ident4():
    """[128, 16] with a 16x16 identity at each 32-partition strip."""
    id4 = np.zeros((128, 16), np.float32)
    for g in range(4):
        id4[32 * g:32 * g + 16, :] = np.eye(16, dtype=np.float32)
    return id4


def _build_bass():
    nc = bacc.Bacc(trn_type="TRN2")
    xin = nc.dram_tensor("xt", [128, NXB * BS], BF16, kind="ExternalInput")
    wsb = nc.dram_tensor("wsb", [128, SLOTS * 128], BF16, kind="ExternalInput")
    dwp = nc.dram_tensor("dwp", [4, NCH * 16], BF16, kind="ExternalInput")
    vecs = nc.dram_tensor("vecs", [3, D], F32, kind="ExternalInput")
    id4 = nc.dram_tensor("ident4", [128, D], F32, kind="ExternalInput")
    out = nc.dram_tensor("out", [BS, D], F32, kind="ExternalOutput")

    with TileContext(nc) as tc:
        with (
            tc.tile_pool(name="const", bufs=1) as const,
            tc.tile_pool(name="cast", bufs=8) as cbuf,
            tc.tile_pool(name="mbuf", bufs=12) as mbuf,
            tc.tile_pool(name="tsegp", bufs=7, space="PSUM") as tsegp,
            tc.tile_pool(name="hpsp", bufs=1, space="PSUM") as hpsp,
            tc.tile_pool(name="lnp", bufs=2) as lnp,
        ):
            # ---- constants / inputs (first pieces sized so window 0 can
            # start as early as possible; spread across issue queues so
            # descriptor generation isn't serialized on one sequencer)
            xT = const.tile([128, NXB, BS], BF16)
            nc.sync.dma_start(out=xT[:], in_=xin[:, :])
            wsb_t = const.tile([128, SLOTS * 128], BF16)
            wcuts = [0, 6 * 128, 16 * 128, 30 * 128, SLOTS * 128]
            for a, b_ in zip(wcuts[:-1], wcuts[1:]):
                nc.sync.dma_start(out=wsb_t[:, a:b_], in_=wsb[:, a:b_])
            # dw2 expansion on-chip: dw2_t[32c+f', k*16+d] = dwp[c, k*16+d]
            # broadcast down 32 partitions per c-group
            dw2_t = const.tile([128, NCH * 16], BF16)
            for c in range(4):
                src = dwp[c:c + 1, :]
                nc.sync.dma_start(
                    out=dw2_t[32 * c:32 * c + 32, :],
                    in_=bass.AP(tensor=src.tensor, offset=src.offset,
                                ap=[[0, 32]] + [list(a) for a in src.ap[1:]]),
                )
            # vecs rows: 0 = dense_b, 1 = gamma, 2 = beta
            vec_t = const.tile([128, 3, D], F32)
            src = vecs[:, :]
            nc.sync.dma_start(
                out=vec_t[:],
                in_=bass.AP(tensor=src.tensor, offset=src.offset,
                            ap=[[0, 128]] + [list(a) for a in src.ap]),
            )
            id4_t = const.tile([128, D], F32)
            nc.sync.dma_start(out=id4_t[:], in_=id4[:, :])
            eps_t = const.tile([128, 1], F32)
            nc.vector.memset(eps_t[:], LN_EPS)
            # dummy Sqrt: loads its activation table during the DMA-bound
            # head instead of inside the LayerNorm tail chain
            warm_t = const.tile([128, 1], F32)
            nc.scalar.activation(
                out=warm_t[:], in_=eps_t[:],
                func=mybir.ActivationFunctionType.Sqrt,
                bias=eps_t[:], scale=1.0,
            )

            # One PSUM bank holds h4 (4 col-tiled partial accumulators
            # [16d x 256b] at partition strips, free 0:256) and hsum
            # ([16d x 256b], free 256:512); ht reuses h4's range later.
            hcomb = hpsp.tile([128, 2 * BS], F32)
            h4 = hcomb[:, 0:BS]
            hsum = hcomb[0:D, BS:2 * BS]
            # zero h4's bank up front: rows outside the four 16-row
            # accumulator strips are never written by dw2 matmuls, and the
            # tail reads the full tile in one copy
            nc.vector.memset(h4[:], 0.0)

            # ---- main pipeline over windows: window w = slots (2w, 2w+1)
            # of each strip.  Each (strip, window) owns a one-bank PSUM pair
            # tile so the 4 strips' stage-1 matmuls of a slot hit 4 distinct
            # banks and run concurrently (row tiling).
            def group_of(r, w):
                slots = [s for s in (2 * w, 2 * w + 1) if s < PER_STRIP[r]]
                return [CH_BY_RS[(r, s)] for s in slots]

            NW = (SLOTS + 1) // 2
            t_tiles = {}     # (r, w) -> psum pair tile
            m_tiles = {}     # (r, w) -> sbuf bf16 pair tile
            pair_idx = 0
            pair_route = {}  # (r, w) -> route

            def stage1(w):
                for u in range(2):
                    for r in range(4):
                        grp = group_of(r, w)
                        if u >= len(grp):
                            continue
                        if u == 0:
                            t_tiles[(r, w)] = tsegp.tile(
                                [128, 2, BS], F32, tag="t", name="tseg")
                        ch = grp[u]
                        s = ch["s"]
                        nc.tensor.matmul(
                            t_tiles[(r, w)][:, u, :],
                            lhsT=wsb_t[32 * r:32 * r + 32,
                                       s * 128:(s + 1) * 128],
                            rhs=xT[32 * r:32 * r + 32, ch["xblk"], :],
                            start=True, stop=True,
                            tile_position=(32 * r, 0),
                        )

            def hadamard(w):
                nonlocal pair_idx
                work = []
                for r in range(4):
                    grp = group_of(r, w)
                    if not grp:
                        continue
                    route = ROUTES[pair_idx]
                    if w >= NW - 2:
                        # drain path: single-op route has the lowest latency
                        route = 1
                    pair_route[(r, w)] = route
                    pair_idx += 1
                    work.append((r, grp, route))
                for r, grp, route in work:
                    tt = t_tiles.pop((r, w))
                    m_t = mbuf.tile([128, 2, BS], BF16, tag="m")
                    m_tiles[(r, w)] = m_t
                    n = len(grp)
                    contig = n == 2 and grp[1]["jg"] == grp[0]["jg"] + 1
                    spans = ([(0, n, grp[0]["jg"])] if (contig or n == 1)
                             else [(0, 1, grp[0]["jg"]), (1, 1, grp[1]["jg"])])
                    for (u0, cnt, jg) in spans:
                        if route == 1:
                            nc.vector.tensor_mul(
                                out=m_t[:, u0:u0 + cnt, :],
                                in0=tt[:, u0:u0 + cnt, :],
                                in1=xT[:, jg:jg + cnt, :])
                        else:
                            tcast = cbuf.tile([128, 2, BS], BF16, tag="tc")
                            nc.scalar.copy(out=tcast[:, u0:u0 + cnt, :],
                                           in_=tt[:, u0:u0 + cnt, :])
                            eng = nc.vector if route == 0 else nc.gpsimd
                            eng.tensor_mul(
                                out=m_t[:, u0:u0 + cnt, :],
                                in0=tcast[:, u0:u0 + cnt, :],
                                in1=xT[:, jg:jg + cnt, :])

            def dw2_fold(w):
                for u in range(2):
                    for r in range(4):
                        grp = group_of(r, w)
                        if u >= len(grp):
                            continue
                        ch = grp[u]
                        k = ch["k"]
                        m_t = m_tiles[(r, w)]
                        nc.tensor.matmul(
                            h4[32 * r:32 * r + D, :],
                            lhsT=dw2_t[:, k * 16:(k + 1) * 16],
                            rhs=m_t[:, u, :],
                            start=(ch["s"] == 0),
                            stop=(ch["s"] == PER_STRIP[r] - 1),
                            tile_position=(0, 32 * r),
                        )
                for r in range(4):
                    m_tiles.pop((r, w), None)

            for w in range(NW + 2):
                if w < NW:
                    stage1(w)
                if 0 <= w - 1 < NW:
                    hadamard(w - 1)
                if 0 <= w - 2 < NW:
                    dw2_fold(w - 2)

            # ---- tail: combine the 4 partial h's with one selector matmul
            # (one full-tile copy; id4's zero rows mask the unused strips)
            hg_sb = lnp.tile([128, BS], F32, tag="hgsb")
            nc.vector.tensor_copy(out=hg_sb[:], in_=h4[:])
            nc.tensor.matmul(hsum, lhsT=id4_t[:], rhs=hg_sb[:],
                             start=True, stop=True)
            hsum_sb = lnp.tile([D, BS], F32, tag="hsum_sb")
            nc.scalar.copy(out=hsum_sb[:], in_=hsum)

            def ht_v(half):
                off = half * D
                return hcomb[0:128, off:off + D]

            for half in range(2):
                nc.tensor.transpose(
                    ht_v(half),
                    hsum_sb[:, half * 128:(half + 1) * 128],
                    id4_t[0:D, :],
                )
            # ---- LayerNorm per 128-row half
            for half in range(2):
                hb = lnp.tile([128, D], F32, tag="hb")
                nc.vector.tensor_add(out=hb[:], in0=ht_v(half),
                                     in1=vec_t[:, 0, :])
                stats = lnp.tile([128, 6], F32, tag="stats")
                nc.vector.bn_stats(out=stats[:], in_=hb[:])
                mv = lnp.tile([128, 2], F32, tag="mv")
                nc.vector.bn_aggr(out=mv[:], in_=stats[:])
                nc.scalar.activation(
                    out=mv[:, 1:2], in_=mv[:, 1:2],
                    func=mybir.ActivationFunctionType.Sqrt,
                    bias=eps_t[:], scale=1.0,
                )
                nc.vector.reciprocal(out=mv[:, 1:2], in_=mv[:, 1:2])
                nc.vector.tensor_scalar(
                    out=hb[:], in0=hb[:],
                    scalar1=mv[:, 0:1], scalar2=mv[:, 1:2],
                    op0=mybir.AluOpType.subtract, op1=mybir.AluOpType.mult,
                )
                nc.vector.tensor_mul(out=hb[:], in0=hb[:], in1=vec_t[:, 1, :])
                nc.vector.tensor_add(out=hb[:], in0=hb[:], in1=vec_t[:, 2, :])
                nc.sync.dma_start(out=out[half * 128:(half + 1) * 128, :],
                                  in_=hb[:])
    nc.finalize()
    return nc


_NC_CACHE = None


def _get_nc():
    global _NC_CACHE
    if _NC_CACHE is None:
        _NC_CACHE = _build_bass()
    return _NC_CACHE


def run(x, W, dense_w, dense_b, gamma, beta, trace=False):
    x = np.asarray(x, np.float32)
    wsb_np, dwp_np = _host_weights(np.asarray(W, np.float32),
                                   np.asarray(dense_w, np.float32))
    vecs_np = np.stack([
        np.asarray(dense_b, np.float32),
        np.asarray(gamma, np.float32),
        np.asarray(beta, np.float32),
    ])
    id4_np = _host_ident4()
    in_maps = []
    for c in range(NCORES):
        in_maps.append({
            "xt": _host_xt(x[c * BS:(c + 1) * BS]),
            "wsb": wsb_np,
            "dwp": dwp_np,
            "vecs": vecs_np,
            "ident4": id4_np,
        })
    res = run_bass_kernel_spmd(
        _get_nc(), in_maps, core_ids=list(range(NCORES)), trace=trace
    )
    out = np.concatenate([res.results[c]["out"] for c in range(NCORES)], axis=0)
    return out.astype(np.float32), res


def kernel(x, W, dense_w, dense_b, gamma, beta):
    out, _ = run(x, W, dense_w, dense_b, gamma, beta)
    return out
